# revision 27
# baseline (speedup 1.0000x reference)
"""Trainium2 Bass kernel for the DynamicBlock (ragged top-k decoder layer).

Sharding: 8 cores = (batch b in 0..3) x (query-half h in 0..1).
Core (b, h) processes queries k in [h*512, (h+1)*512) of the K=1024 selected
rows of batch b (causal: needs K/V for all 1024 selected rows, computed
locally -- no collectives).  Untouched hidden rows are assembled host-side.
Matmuls run in bf16 with fp32 accumulation; norms/softmax/residual/gating
in fp32.

Attention exploits causality uniformly across cores: keys are permuted so
this core's own query half is tiles j=0..3 (block-triangular: tile j only
affects query columns >= j*128) and the other half is tiles j=4..7, which
are either fully visible (h=1) or fully masked (h=0) -- expressed as a
per-core additive bias on the exp, so one program serves both core types.
"""

import math
from contextlib import ExitStack
from dataclasses import dataclass

import ml_dtypes
import numpy as np

import concourse.bass as bass
import concourse.mybir as mybir
import concourse.tile as tile
from concourse import bacc
from concourse.bass import IndirectOffsetOnAxis

P = 128
F32 = mybir.dt.float32
BF16 = mybir.dt.bfloat16
F8 = mybir.dt.float8e4
I32 = mybir.dt.int32
AF = mybir.ActivationFunctionType
BF16NP = ml_dtypes.bfloat16
F8NP = ml_dtypes.float8_e4m3
DR = mybir.MatmulPerfMode.DoubleRow


@dataclass(frozen=True)
class Cfg:
    T: int = 4096      # full sequence length
    D: int = 2048      # model dim
    KSEL: int = 1024   # selected rows per sequence
    H: int = 16        # query heads
    KVH: int = 4       # kv heads
    HD: int = 128      # head dim (must equal P)
    FF: int = 8192     # mlp intermediate
    EPS: float = 1e-6

    @property
    def DT(self):
        return self.D // P

    @property
    def QROWS(self):
        return self.KSEL // 2

    @property
    def QT(self):
        return self.QROWS // P

    @property
    def KT(self):
        return self.KSEL // P

    @property
    def FFT(self):
        return self.FF // P

    @property
    def FFG(self):
        return self.FFT // 4

    @property
    def T2(self):
        return self.T // 2

    @property
    def GQ(self):
        return self.H // self.KVH


FULL = Cfg()


def _chunks(total, size):
    out = []
    s = 0
    while s < total:
        out.append((s, min(size, total - s)))
        s += size
    return out


def emit(nc: bass.Bass, c: Cfg, upto: str = "G"):
    _PH = "ABCDEFG"

    def _ph(p):
        return _PH.index(p) <= _PH.index(upto)

    DT, QT, KT, QROWS, KVD = c.DT, c.QT, c.KT, c.QROWS, c.KVH * c.HD
    OGS = _chunks(c.D, 512)  # output-column groups for o-proj / down-proj
    OW = OGS[0][1]

    # ---- DRAM I/O ----
    hid_d = nc.dram_tensor("hid", [c.T, c.D], F32, kind="ExternalInput")
    idxkv_d = nc.dram_tensor("idx_kv", [P, KT], I32, kind="ExternalInput")
    gsc_d = nc.dram_tensor("gsc", [P, QT], F32, kind="ExternalInput")
    cos_d = nc.dram_tensor("cosb", [c.T, c.HD], F32, kind="ExternalInput")
    sin_d = nc.dram_tensor("sinb", [c.T, c.HD], F32, kind="ExternalInput")
    biasj_d = nc.dram_tensor("biasj", [P, KT], F32, kind="ExternalInput")
    tri_d = nc.dram_tensor("tri", [P, P], F8, kind="ExternalInput")
    wq_d = nc.dram_tensor("wq", [c.H, P, DT, c.HD], BF16, kind="ExternalInput")
    wk_d = nc.dram_tensor("wk", [c.KVH, P, DT, c.HD], BF16, kind="ExternalInput")
    wv_d = nc.dram_tensor("wv", [P, DT, KVD], BF16, kind="ExternalInput")
    wo_d = nc.dram_tensor("wo", [len(OGS), c.H // 4, P, 4, OW], BF16, kind="ExternalInput")
    wg_d = nc.dram_tensor("wg", [c.FFG, DT // 4, P, 4, 512], BF16, kind="ExternalInput")
    wu_d = nc.dram_tensor("wu", [c.FFG, DT // 4, P, 4, 512], BF16, kind="ExternalInput")
    wd_d = nc.dram_tensor("wd", [len(OGS), c.FFT // 4, P, 4, OW], BF16, kind="ExternalInput")
    bq_d = nc.dram_tensor("bq", [c.H, P, 1], F32, kind="ExternalInput")
    bk_d = nc.dram_tensor("bk", [c.KVH, P, 1], F32, kind="ExternalInput")
    bv_d = nc.dram_tensor("bv", [1, KVD], F32, kind="ExternalInput")
    idf_d = nc.dram_tensor("id_f", [P, P], F32, kind="ExternalInput")
    idb_d = nc.dram_tensor("id_b", [P, P], BF16, kind="ExternalInput")
    perm_d = nc.dram_tensor("perm", [P, P], BF16, kind="ExternalInput")
    ones_d = nc.dram_tensor("ones16", [P, 2, P], F8, kind="ExternalInput")

    oupd_d = nc.dram_tensor("out_upd", [QROWS, c.D], F32, kind="ExternalOutput")

    scl = 1.0 / math.sqrt(c.HD)

    with ExitStack() as top:
        tc = top.enter_context(tile.TileContext(nc))
        constp = top.enter_context(tc.tile_pool(name="constp", bufs=1, side="left"))
        residp = top.enter_context(tc.tile_pool(name="residp", bufs=1, side="left"))
        psp = top.enter_context(tc.tile_pool(name="psp", bufs=6, space="PSUM"))
        psbp = top.enter_context(tc.tile_pool(name="psbp", bufs=2, space="PSUM"))

        def ps_tile():
            return psp.tile([P, 512], F32, tag="ps", name="ps")

        def psb_tile():
            return psbp.tile([P, P], BF16, tag="psb", name="psb")

        # ---- constants (indices first: they gate the gathers) ----
        idxkv = constp.tile([P, KT], I32, tag="idxkv")
        nc.sync.dma_start(idxkv[:], idxkv_d[:])
        idf = constp.tile([P, P], F32, tag="idf")
        nc.sync.dma_start(idf[:], idf_d[:])
        idb = constp.tile([P, P], BF16, tag="idb")
        nc.sync.dma_start(idb[:], idb_d[:])
        perm = constp.tile([P, P], BF16, tag="perm")
        nc.sync.dma_start(perm[:], perm_d[:])
        ones16 = constp.tile([P, 2, P], F8, tag="ones16")
        nc.sync.dma_start(ones16[:], ones_d[:])
        tri = constp.tile([P, P], F8, tag="tri")
        nc.sync.dma_start(tri[:], tri_d[:])
        gsc = constp.tile([P, QT], F32, tag="gsc")
        nc.sync.dma_start(gsc[:], gsc_d[:])
        biasjc = constp.tile([P, KT], F32, tag="biasjc")
        nc.sync.dma_start(biasjc[:], biasj_d[:])
        bqc = constp.tile([P, c.H], F32, tag="bqc")
        for m in range(c.H):
            nc.sync.dma_start(bqc[:, m : m + 1], bq_d[m])
        bkc = constp.tile([P, c.KVH], F32, tag="bkc")
        for m in range(c.KVH):
            nc.sync.dma_start(bkc[:, m : m + 1], bk_d[m])
        epsc = constp.tile([P, 1], F32, tag="epsc")
        nc.vector.memset(epsc[:], c.EPS)
        bvbc = constp.tile([P, KVD], F32, tag="bvbc")
        bv_ap = bv_d[:]
        nc.sync.dma_start(
            bvbc[:], bass.AP(tensor=bv_ap.tensor, offset=0, ap=[[0, P], [1, KVD]])
        )

        # residual (live until the end)
        xq_raw = residp.tile([P, QT, c.D], F32, tag="xq_raw")

        sgw = math.gcd(512, c.D)
        nsub = c.D // sgw

        es_bt = ExitStack()  # xkvT/xqT/cos/sin: freed after projections
        xtp = es_bt.enter_context(tc.tile_pool(name="xtp", bufs=1, side="left"))
        xkvT = xtp.tile([P, DT, c.KSEL], BF16, tag="xkvT")
        cosTkv = xtp.tile([P, c.KSEL], F32, tag="cosTkv")
        sinTkv = xtp.tile([P, c.KSEL], F32, tag="sinTkv")
        # host permutes the key order so this core's own query half is rows
        # [0, QROWS) -- q-side tensors are static slices of the kv tensors
        xqT = xkvT[:, :, :QROWS]
        cosTq = cosTkv[:, :QROWS]
        sinTq = sinTkv[:, :QROWS]

        # ---- phase B: gather + rmsnorm1 + transpose ----
        def gather_rows(dst, src_dram, idx_tile, col):
            """Indirect row gather with a gpsimd shield op.

            The shield write/read absorbs the WAR (slot reuse) and RAW (index
            load) waits into a compute op on the triggering engine -- the
            dynamic-queue DMA itself only supports a single sync wait.
            """
            nc.gpsimd.tensor_copy(dst[0:1, 0:1], idx_tile[0:1, col : col + 1])
            nc.gpsimd.indirect_dma_start(
                out=dst,
                out_offset=None,
                in_=src_dram[:],
                in_offset=IndirectOffsetOnAxis(ap=idx_tile[:, col : col + 1], axis=0),
            )

        def norm_transpose(raw, xn_out_fn):
            """raw: [P, D] f32 tile; writes bf16 normalized transposed tiles.

            RMSNorm needs only E[x^2]: one tensor_tensor_reduce (x*x with an
            add-reduction) replaces the bn_stats/bn_aggr chain.
            """
            stats = spool.tile([P, nsub, 6], F32, tag="stats")
            for s in range(nsub):
                nc.vector.bn_stats(stats[:, s, :], raw[:, s * sgw : (s + 1) * sgw])
            mv = spool.tile([P, 2], F32, tag="mv")
            nc.vector.bn_aggr(mv[:], stats[:])
            msq = spool.tile([P, 1], F32, tag="msq")
            nc.vector.tensor_mul(msq[:], mv[:, 0:1], mv[:, 0:1])
            nc.vector.tensor_add(msq[:], msq[:], mv[:, 1:2])
            srt = spool.tile([P, 1], F32, tag="srt")
            nc.scalar.activation(srt[:], msq[:], AF.Sqrt, bias=epsc[:])
            rstd = spool.tile([P, 1], F32, tag="rstd")
            nc.vector.reciprocal(rstd[:], srt[:])
            xn = gpool.tile([P, c.D], BF16, tag="xn")
            nc.vector.tensor_scalar_mul(xn[:], raw[:], rstd[:])
            for dt in range(DT):
                tp = psb_tile()
                nc.tensor.transpose(tp[:], xn[:, dt * P : (dt + 1) * P], idb[:])
                nc.scalar.copy(xn_out_fn(dt), tp[:])

        with tc.tile_pool(name="gpool", bufs=4, side="left") as gpool, tc.tile_pool(name="spool", bufs=4, side="left") as spool, tc.tile_pool(name="cpool", bufs=3, side="left") as cpool:
            for t in range(KT if _ph("B") else 0):
                if t < QT:
                    raw = xq_raw[:, t, :]
                else:
                    raw = gpool.tile([P, c.D], F32, tag="graw", name="graw")[:]
                gather_rows(raw, hid_d, idxkv, t)
                norm_transpose(
                    raw, lambda dt, t=t: xkvT[:, dt, t * P : (t + 1) * P]
                )
            # cos/sin gathers + transposes (f32)
            for srcd, idxt, nt, dst in () if not _ph("B") else (
                (cos_d, idxkv, KT, cosTkv),
                (sin_d, idxkv, KT, sinTkv),
            ):
                for t in range(nt):
                    cg = cpool.tile([P, c.HD], F32, tag="cg")
                    gather_rows(cg[:], srcd, idxt, t)
                    tp = ps_tile()
                    nc.tensor.transpose(tp[:, :P], cg[:], idf[:])
                    nc.scalar.copy(dst[:, t * P : (t + 1) * P], tp[:, :P])

        # ---- phases C+D: projections + rope, interleaved with attention ----
        # Emission order: K proj, V proj, Q(0), Q(1), then per attention head
        # h: Q(h+2) and scores(h+1) are emitted BEFORE AV/ones(h), so the PE
        # queue always has dense independent matmuls to run while the
        # exp->mask chain of the current head completes on scalar/vector.
        es_qkv = ExitStack()
        qkvp = es_qkv.enter_context(tc.tile_pool(name="qkvp", bufs=1, side="right"))
        kT = qkvp.tile([P, c.KVH, c.KSEL], BF16, tag="kT")
        vN = qkvp.tile([P, KT, KVD], BF16, tag="vN")
        qT = qkvp.tile([P, c.H, QROWS], BF16, tag="qT")

        # o-proj weight prefetch: fresh SBUF region + early queue position so
        # chunks stream in during C/D and phase E never waits on weights
        es_wo = ExitStack()
        wop = es_wo.enter_context(tc.tile_pool(name="wop", bufs=4, side="left"))
        wo_tiles = {}
        for ogi in range(len(OGS) if _ph("E") else 0):
            for c4 in range(c.H // 4):
                wt = wop.tile([P, 4, OW], BF16, tag="wot")
                # gpsimd-engine queue: decoupled from the sync queue so these
                # prefetches never head-of-line-block the Q/K weight streams
                nc.gpsimd.dma_start(wt[:], wo_d[ogi, c4])
                wo_tiles[(ogi, c4)] = wt

        es_attn = ExitStack()
        attnp = es_attn.enter_context(tc.tile_pool(name="attnp", bufs=1, side="left"))
        xattnT = attnp.tile([P, c.H, QROWS], BF16, tag="xattnT")

        # Causal structure (keys permuted: own half first):
        #   tile j<4: affects only query cols >= j*128; diagonal 128x128
        #     sub-block is triangular (tri multiply); rest fully visible.
        #   tile j>=4: all-visible (h=1) or all-masked (h=0) -- via biasj.
        # exp computes exp(scl*s + biasj) with biasj in {-1, -30001}: the
        # uniform -1 shift guards fp8 overflow and cancels in the divide.
        # expT is fp8 (softmax weights in [0, e^4.5]); the softmax-sum uses a
        # DoubleRow fp8 matmul against a constant 16.0 stationary (the x16
        # is divided back out in the final normalization).
        def jow(j):
            o = j * P if j < 4 else 0
            return o, QROWS - o

        with tc.tile_pool(name="wstr", bufs=3, side="left") as wstr, tc.tile_pool(name="rpool", bufs=3, side="left") as rpool, tc.tile_pool(name="ropep", bufs=2, side="left") as ropep, tc.tile_pool(name="dpool", bufs=3, side="left") as dpool, tc.tile_pool(name="recp", bufs=3, side="left") as recp:

            def rope(dst, rawt, rot_ps, cosT, sinT, s0, w):
                t1 = ropep.tile([P, 512], F32, tag="ropet1")
                nc.vector.tensor_mul(t1[:, :w], rawt[:, s0 : s0 + w], cosT[:, s0 : s0 + w])
                t2 = ropep.tile([P, 512], F32, tag="ropet2")
                nc.vector.tensor_mul(t2[:, :w], rot_ps[:, :w], sinT[:, s0 : s0 + w])
                nc.vector.tensor_add(dst[:, s0 : s0 + w], t1[:, :w], t2[:, :w])

            def qproj(m):
                wqm = wstr.tile([P, DT, c.HD], BF16, tag="wqkm", name="wqm")
                nc.sync.dma_start(wqm[:], wq_d[m])
                qraw = rpool.tile([P, c.KSEL], BF16, tag="kqraw", name="qraw")
                ps = ps_tile()
                for dt in range(DT):
                    nc.tensor.matmul(
                        ps[:, :QROWS],
                        wqm[:, dt, :],
                        xqT[:, dt, :],
                        start=(dt == 0),
                        stop=(dt == DT - 1),
                    )
                nc.vector.tensor_scalar_add(
                    qraw[:, :QROWS], ps[:, :QROWS], bqc[:, m : m + 1]
                )
                rot = ps_tile()
                nc.tensor.matmul(
                    rot[:, :QROWS], perm[:], qraw[:, :QROWS], start=True, stop=True
                )
                rope(qT[:, m, :], qraw, rot, cosTq, sinTq, 0, QROWS)

            def kproj(m):
                wkm = wstr.tile([P, DT, c.HD], BF16, tag="wqkm", name="wkm")
                nc.sync.dma_start(wkm[:], wk_d[m])
                kraw = rpool.tile([P, c.KSEL], BF16, tag="kqraw")
                for s0, w in _chunks(c.KSEL, 512):
                    ps = ps_tile()
                    for dt in range(DT):
                        nc.tensor.matmul(
                            ps[:, :w],
                            wkm[:, dt, :],
                            xkvT[:, dt, s0 : s0 + w],
                            start=(dt == 0),
                            stop=(dt == DT - 1),
                        )
                    nc.vector.tensor_scalar_add(
                        kraw[:, s0 : s0 + w], ps[:, :w], bkc[:, m : m + 1]
                    )
                for s0, w in _chunks(c.KSEL, 512):
                    rot = ps_tile()
                    nc.tensor.matmul(
                        rot[:, :w], perm[:], kraw[:, s0 : s0 + w], start=True, stop=True
                    )
                    rope(kT[:, m, :], kraw, rot, cosTkv, sinTkv, s0, w)

            exp_tiles = {}

            def scores_block(h):
                g = h // c.GQ
                expT = dpool.tile([P, KT, QROWS], F8, tag="expT")
                exp_tiles[h] = expT
                # zero the fp8 pair-mate gaps (cols outside a tile's causal
                # range that its DoubleRow partner still streams)
                nc.gpsimd.memset(expT[:, 1, 0:P], 0.0)
                nc.gpsimd.memset(expT[:, 3, 2 * P : 3 * P], 0.0)
                for j in range(KT):
                    o, w = jow(j)
                    ps = ps_tile()
                    nc.tensor.matmul(
                        ps[:, o:QROWS],
                        kT[:, g, j * P : (j + 1) * P],
                        qT[:, h, o:QROWS],
                        start=True,
                        stop=True,
                    )
                    nc.scalar.activation(
                        expT[:, j, o:QROWS], ps[:, o:QROWS], AF.Exp,
                        scale=scl, bias=biasjc[:, j : j + 1],
                    )
                    if j < 4:
                        nc.vector.tensor_mul(
                            expT[:, j, j * P : (j + 1) * P],
                            expT[:, j, j * P : (j + 1) * P],
                            tri[:],
                        )

            def av_ones_drain(h):
                g = h // c.GQ
                expT = exp_tiles.pop(h)
                pso = ps_tile()
                pss = ps_tile()
                for j in range(KT):
                    o, w = jow(j)
                    nc.tensor.matmul(
                        pso[:, o:QROWS],
                        vN[:, j, g * c.HD : (g + 1) * c.HD],
                        expT[:, j, o:QROWS],
                        start=(j == 0),
                        stop=(j == KT - 1),
                        skip_group_check=True,
                    )
                for p2 in range(KT // 2):
                    o, w = jow(2 * p2)
                    nc.tensor.matmul(
                        pss[:, o:QROWS],
                        ones16[:],
                        expT[:, 2 * p2 : 2 * p2 + 2, o:QROWS],
                        start=(p2 == 0),
                        stop=(p2 == KT // 2 - 1),
                        skip_group_check=True,
                        perf_mode=DR,
                    )
                rec = recp.tile([P, QROWS], F32, tag="rec")
                nc.vector.reciprocal_approx_fast(rec[:], pss[:, :QROWS])
                # pss = 16*sum(exp): fold the /16 back via the 16.0 scalar
                nc.vector.scalar_tensor_tensor(
                    xattnT[:, h, :], pso[:, :QROWS], 16.0, rec[:],
                    mybir.AluOpType.mult, mybir.AluOpType.mult,
                )

            # K + V first, then Q, then attention (scores one head ahead)
            with tc.tile_pool(name="wvp", bufs=1, side="left") as wvp:
                wvsb = wvp.tile([P, DT, KVD], BF16, tag="wvsb")
                nc.sync.dma_start(wvsb[:], wv_d[:])
                for m in range(c.KVH if _ph("C") else 0):
                    kproj(m)
                for rt in range(KT if _ph("C") else 0):
                    psv = ps_tile()
                    for dt in range(DT):
                        nc.tensor.matmul(
                            psv[:, :KVD],
                            xkvT[:, dt, rt * P : (rt + 1) * P],
                            wvsb[:, dt, :],
                            start=(dt == 0),
                            stop=(dt == DT - 1),
                        )
                    nc.vector.tensor_add(vN[:, rt, :], psv[:, :KVD], bvbc[:])

            for m in range(c.H if _ph("C") else 0):
                qproj(m)

            if _ph("D"):
                scores_block(0)
                for h in range(c.H):
                    if h + 1 < c.H:
                        scores_block(h + 1)
                    av_ones_drain(h)

        es_qkv.close()  # free kT/vN/qT (right side)

        # attn+mlp residual, lives E -> G
        es_res2 = ExitStack()
        res2p = es_res2.enter_context(tc.tile_pool(name="res2p", bufs=1, side="right"))
        res2 = res2p.tile([P, QT, c.D], F32, tag="res2")

        # ---- phase E: o-proj + residual + rmsnorm2 ----
        es_xm = ExitStack()
        xmp = es_xm.enter_context(tc.tile_pool(name="xmp", bufs=1, side="right"))
        xmT = xmp.tile([P, DT, QROWS], BF16, tag="xmT")

        with tc.tile_pool(name="gpool2", bufs=3, side="left") as gpool2, tc.tile_pool(name="spool2", bufs=4, side="left") as spool2:
            def _norm2_transpose(qt):
                mv = spool2.tile([P, 2], F32, tag="mv2", name="mv")
                nc.vector.bn_aggr(mv[:], stats2[:, qt])
                msq = spool2.tile([P, 1], F32, tag="msq2", name="msq")
                nc.vector.tensor_mul(msq[:], mv[:, 0:1], mv[:, 0:1])
                nc.vector.tensor_add(msq[:], msq[:], mv[:, 1:2])
                srt = spool2.tile([P, 1], F32, tag="srt2", name="srt")
                nc.scalar.activation(srt[:], msq[:], AF.Sqrt, bias=epsc[:])
                rstd = spool2.tile([P, 1], F32, tag="rstd2", name="rstd")
                nc.vector.reciprocal(rstd[:], srt[:])
                xn = gpool2.tile([P, c.D], BF16, tag="xn2", name="xn")
                nc.vector.tensor_scalar_mul(xn[:], res2[:, qt, :], rstd[:])
                for dt in range(DT):
                    tp = psb_tile()
                    nc.tensor.transpose(
                        tp[:], xn[:, dt * P : (dt + 1) * P], idb[:]
                    )
                    nc.scalar.copy(xmT[:, dt, qt * P : (qt + 1) * P], tp[:])

            stats2 = spool2.tile([P, QT, nsub, 6], F32, tag="stats2all")
            for ogi, (os_, ow) in enumerate(OGS if _ph("E") else []):
                pss4 = [ps_tile() for _ in range(QT)]
                for c4 in range(c.H // 4):
                    wot = wo_tiles[(ogi, c4)]
                    for i in range(4):
                        ht = c4 * 4 + i
                        for qt in range(QT):
                            nc.tensor.matmul(
                                pss4[qt][:, :ow],
                                xattnT[:, ht, qt * P : (qt + 1) * P],
                                wot[:, i, :ow],
                                start=(ht == 0),
                                stop=(ht == c.H - 1),
                            )
                for qt in range(QT):
                    nc.vector.tensor_add(
                        res2[:, qt, os_ : os_ + ow],
                        pss4[qt][:, :ow],
                        xq_raw[:, qt, os_ : os_ + ow],
                    )
                    # incremental norm2 stats: OGS chunks == bn subgroups
                    nc.vector.bn_stats(
                        stats2[:, qt, ogi, :], res2[:, qt, os_ : os_ + ow]
                    )
            for qt in range(QT if _ph("E") else 0):
                _norm2_transpose(qt)
            for qt in range(QT if _ph("E") else 0):
                # fold gating: res2 := (res2 - xq_raw)*g + xq_raw, so phase G
                # only needs one fused op per output tile after the last MM
                nc.vector.tensor_sub(
                    res2[:, qt, :], res2[:, qt, :], xq_raw[:, qt, :]
                )
                nc.vector.tensor_scalar_mul(
                    res2[:, qt, :], res2[:, qt, :], gsc[:, qt : qt + 1]
                )
                nc.vector.tensor_add(
                    res2[:, qt, :], res2[:, qt, :], xq_raw[:, qt, :]
                )

        es_attn.close()  # free xattnT
        es_wo.close()    # free wo tiles
        es_bt.close()    # free xkvT/xqT/cos/sin (held through D for Q-proj)

        # ---- phase F: mlp gate/up ----
        es_act = ExitStack()
        actp = es_act.enter_context(tc.tile_pool(name="actp", bufs=1, side="left"))
        actT = actp.tile([P, c.FFT, QROWS], BF16, tag="actT")

        es_wd = ExitStack()  # down-proj weight stream: spans F (prefetch) + G
        wstr4 = es_wd.enter_context(tc.tile_pool(name="wstr4", bufs=6, side="left"))
        wd_pre = []
        for pi in range(2 if _ph("G") else 0):
            wdt = wstr4.tile([P, 4, OW], BF16, tag="wdt")
            nc.gpsimd.dma_start(wdt[:], wd_d[0, pi])
            wd_pre.append(wdt)

        with tc.tile_pool(name="wstr3", bufs=4, side="left") as wstr3, tc.tile_pool(name="fpool", bufs=2, side="left") as fpool:
            for g in range(c.FFG if _ph("F") else 0):
                psg = [ps_tile() for _ in range(4)]
                for d4 in range(DT // 4):
                    wgt = wstr3.tile([P, 4, 512], BF16, tag="wgut")
                    nc.sync.dma_start(wgt[:], wg_d[g, d4])
                    for i in range(4):
                        dt = d4 * 4 + i
                        for s in range(4):
                            nc.tensor.matmul(
                                psg[s][:, :QROWS],
                                wgt[:, i, s * P : (s + 1) * P],
                                xmT[:, dt, :],
                                start=(dt == 0),
                                stop=(dt == DT - 1),
                            )
                silu = fpool.tile([P, 4, QROWS], F32, tag="silu")
                for s in range(4):
                    # silu(x) = x * sigmoid(x) (Silu isn't in CoreSim)
                    nc.scalar.activation(silu[:, s, :], psg[s][:, :QROWS], AF.Sigmoid)
                    nc.vector.tensor_mul(silu[:, s, :], silu[:, s, :], psg[s][:, :QROWS])
                psu = [ps_tile() for _ in range(4)]
                for d4 in range(DT // 4):
                    wut = wstr3.tile([P, 4, 512], BF16, tag="wgut")
                    nc.sync.dma_start(wut[:], wu_d[g, d4])
                    for i in range(4):
                        dt = d4 * 4 + i
                        for s in range(4):
                            nc.tensor.matmul(
                                psu[s][:, :QROWS],
                                wut[:, i, s * P : (s + 1) * P],
                                xmT[:, dt, :],
                                start=(dt == 0),
                                stop=(dt == DT - 1),
                            )
                for s in range(4):
                    nc.vector.tensor_mul(
                        actT[:, g * 4 + s, :], silu[:, s, :], psu[s][:, :QROWS]
                    )

        es_xm.close()  # free xmT

        # ---- phase G: down-proj + residual + gating + output ----
        with tc.tile_pool(name="opool", bufs=3, side="left") as opool:
            for ogi, (os_, ow) in enumerate(OGS if _ph("G") else []):
                psd = [ps_tile() for _ in range(QT)]
                for f4 in range(c.FFT // 4):
                    if ogi == 0 and f4 < 2:
                        wdt = wd_pre[f4]
                    else:
                        wdt = wstr4.tile([P, 4, OW], BF16, tag="wdt")
                        nc.sync.dma_start(wdt[:], wd_d[ogi, f4])
                    for i in range(4):
                        ffp = f4 * 4 + i
                        for qt in range(QT):
                            nc.tensor.matmul(
                                psd[qt][:, :ow],
                                actT[:, ffp, qt * P : (qt + 1) * P],
                                wdt[:, i, :ow],
                                start=(ffp == 0),
                                stop=(ffp == c.FFT - 1),
                            )
                for qt in range(QT):
                    t1 = opool.tile([P, 512], F32, tag="updt")
                    nc.vector.scalar_tensor_tensor(
                        t1[:, :ow],
                        psd[qt][:, :ow],
                        gsc[:, qt : qt + 1],
                        res2[:, qt, os_ : os_ + ow],
                        mybir.AluOpType.mult,
                        mybir.AluOpType.add,
                    )
                    nc.sync.dma_start(
                        oupd_d[qt * P : (qt + 1) * P, os_ : os_ + ow], t1[:, :ow]
                    )

        es_wd.close()
        es_act.close()
        es_res2.close()
    return nc


# ---------------- host side ----------------


def _bf(x):
    return np.ascontiguousarray(x.astype(BF16NP))


def _f32(x):
    return np.ascontiguousarray(x, dtype=np.float32)


def prep_shared(c: Cfg, Wq, bq, Wk, bk, Wv, bv, Wo, w_gate, w_up, w_down, ln1_w, ln2_w):
    """Host-side weight folding + tiling (exact fp32 math, then bf16 cast)."""
    DT, FFT, FFG, KVD = c.DT, c.FFT, c.FFG, c.KVH * c.HD
    OGS = _chunks(c.D, 512)
    OG, OW = len(OGS), OGS[0][1]
    Wqf = _f32(Wq) * _f32(ln1_w)[:, None]
    Wkf = _f32(Wk) * _f32(ln1_w)[:, None]
    Wvf = _f32(Wv) * _f32(ln1_w)[:, None]
    Wgf = _f32(w_gate) * _f32(ln2_w)[:, None]
    Wuf = _f32(w_up) * _f32(ln2_w)[:, None]

    perm = np.zeros((P, P), np.float32)
    half = c.HD // 2
    perm[np.arange(half) + half, np.arange(half)] = -1.0
    perm[np.arange(half), np.arange(half) + half] = 1.0

    # tri[k, q] = 1 if k <= q (keep) else 0, for the diagonal 128x128 block
    tri = np.triu(np.ones((P, P), np.float32))

    return dict(
        wq=_bf(Wqf.reshape(DT, P, c.H, c.HD).transpose(2, 1, 0, 3)),
        wk=_bf(Wkf.reshape(DT, P, c.KVH, c.HD).transpose(2, 1, 0, 3)),
        wv=_bf(Wvf.reshape(DT, P, KVD).transpose(1, 0, 2)),
        # wo[ogi, c4, p, i, col] = Wo[(c4*4+i)*128+p, ogi*512+col]
        wo=_bf(_f32(Wo).reshape(c.H // 4, 4, P, OG, OW).transpose(3, 0, 2, 1, 4)),
        # wg[g, d4, p, i, col] = Wgf[(d4*4+i)*128+p, g*512+col]
        wg=_bf(Wgf.reshape(DT // 4, 4, P, FFG, 512).transpose(3, 0, 2, 1, 4)),
        wu=_bf(Wuf.reshape(DT // 4, 4, P, FFG, 512).transpose(3, 0, 2, 1, 4)),
        # wd[ogi, f4, p, i, col] = w_down[(f4*4+i)*128+p, ogi*512+col]
        wd=_bf(_f32(w_down).reshape(FFT // 4, 4, P, OG, OW).transpose(3, 0, 2, 1, 4)),
        bq=_f32(bq).reshape(c.H, P, 1),
        bk=_f32(bk).reshape(c.KVH, P, 1),
        bv=_f32(bv).reshape(1, KVD),
        id_f=np.eye(P, dtype=np.float32),
        id_b=np.eye(P, dtype=np.float32).astype(BF16NP),
        perm=perm.astype(BF16NP),
        ones16=np.full((P, 2, P), 16.0, np.float32).astype(F8NP),
        tri=tri.astype(F8NP),
    )


def prep_core(c: Cfg, shared, hid_b, idx_b, g_b, cos_b, sin_b, h):
    """Per-core inputs for core handling query-half h of one batch."""
    QROWS, QT, KT = c.QROWS, c.QT, c.KT
    idx32 = idx_b.astype(np.int32)
    # permute keys so this core's own query half comes first; block-causal
    # masking is then uniform: tile j<4 affects only cols >= j*128 with a
    # triangular diagonal block; tiles j>=4 are all-or-nothing via biasj
    kperm = np.concatenate(
        [np.arange(h * QROWS, (h + 1) * QROWS),
         np.arange(0, h * QROWS), np.arange((h + 1) * QROWS, c.KSEL)]
    )
    idx32 = idx32[kperm]
    # exp bias per key tile: -1 everywhere (overflow guard, cancels in the
    # normalization); other-half tiles fully masked for h=0 cores
    biasj = np.full((P, KT), -1.0, np.float32)
    if h == 0:
        biasj[:, 4:] = -30001.0
    m = dict(
        hid=_f32(hid_b),
        idx_kv=np.ascontiguousarray(idx32.reshape(KT, P).T),
        gsc=np.ascontiguousarray(
            _f32(g_b[h * QROWS : (h + 1) * QROWS]).reshape(QT, P).T
        ),
        cosb=_f32(cos_b),
        sinb=_f32(sin_b),
        biasj=biasj,
    )
    m.update(shared)
    return m


_NC_CACHE = {}


def _get_nc(c: Cfg):
    key = c
    if key not in _NC_CACHE:
        nc = bacc.Bacc()
        emit(nc, c)
        nc.compile()
        _NC_CACHE[key] = nc
    return _NC_CACHE[key]


_RUN_CACHE = {}


def _run_spmd_cached(c: Cfg, nc, in_maps):
    """run_bass_via_pjrt equivalent with a cached jitted executable.

    run_bass_kernel_spmd rebuilds its jit closure per call, so every kernel()
    invocation would re-trace + recompile (~40s).  Build the shard_map jit
    once per config and reuse it; repeat calls only pay host->device
    transfer + execution.
    """
    import jax
    import numpy as np
    from jax.sharding import Mesh, PartitionSpec
    from jax.experimental.shard_map import shard_map
    from concourse import bass2jax
    from concourse.bass2jax import _bass_exec_p, install_neuronx_cc_hook

    n_cores = len(in_maps)
    key = (c, n_cores)
    if key not in _RUN_CACHE:
        install_neuronx_cc_hook()
        partition_name = (
            nc.partition_id_tensor.name if nc.partition_id_tensor else None
        )
        in_names, out_names, out_avals = [], [], []
        for alloc in nc.m.functions[0].allocations:
            if not isinstance(alloc, mybir.MemoryLocationSet):
                continue
            name = alloc.memorylocations[0].name
            if alloc.kind == "ExternalInput":
                if name != partition_name:
                    in_names.append(name)
            elif alloc.kind == "ExternalOutput":
                out_names.append(name)
                out_avals.append(
                    jax.core.ShapedArray(
                        tuple(alloc.tensor_shape), mybir.dt.np(alloc.dtype)
                    )
                )
        n_params = len(in_names)
        all_in = list(in_names) + list(out_names)
        if partition_name is not None:
            all_in.append(partition_name)

        def _body(*flat):
            operands = list(flat)
            if partition_name is not None:
                operands.append(bass2jax.partition_id_tensor())
            return tuple(
                _bass_exec_p.bind(
                    *operands,
                    out_avals=tuple(out_avals),
                    in_names=tuple(all_in),
                    out_names=tuple(out_names),
                    lowering_input_output_aliases=(),
                    sim_require_finite=True,
                    sim_require_nnan=True,
                    nc=nc,
                )
            )

        devices = jax.devices()[:n_cores]
        mesh = Mesh(np.asarray(devices), ("core",))
        n_outs = len(out_avals)
        sharded = jax.jit(
            shard_map(
                _body,
                mesh=mesh,
                in_specs=(PartitionSpec("core"),) * (n_params + n_outs),
                out_specs=(PartitionSpec("core"),) * n_outs,
                check_rep=False,
            ),
            keep_unused=True,
        )
        zeros = [
            np.zeros((n_cores * a.shape[0], *a.shape[1:]), a.dtype)
            for a in out_avals
        ]
        _RUN_CACHE[key] = (sharded, in_names, out_names, out_avals, zeros)

    sharded, in_names, out_names, out_avals, zeros = _RUN_CACHE[key]
    concat_in = [
        np.concatenate([np.asarray(in_maps[ci][nm]) for ci in range(n_cores)], axis=0)
        for nm in in_names
    ]
    out_arrs = sharded(*concat_in, *zeros)
    return [
        {
            name: np.asarray(out_arrs[i]).reshape(n_cores, *out_avals[i].shape)[ci]
            for i, name in enumerate(out_names)
        }
        for ci in range(n_cores)
    ]


def assemble_output(inputs, res):
    """Build the full [B, T, D] output from per-core result maps."""
    c = FULL
    hidden_states = np.asarray(inputs["hidden_states"])
    topk = np.asarray(inputs["topk_indices"])
    B = hidden_states.shape[0]
    final = np.ascontiguousarray(hidden_states, dtype=np.float32).copy()
    for ci in range(2 * B):
        b, h = ci // 2, ci % 2
        sel = topk[b, h * c.QROWS : (h + 1) * c.QROWS].astype(np.int64)
        final[b, sel] = res[ci]["out_upd"]
    return final


def kernel(
    hidden_states,
    topk_indices,
    gating_scores,
    cos,
    sin,
    Wq,
    bq,
    Wk,
    bk,
    Wv,
    bv,
    Wo,
    w_gate,
    w_up,
    w_down,
    ln1_w,
    ln2_w,
):
    c = FULL
    B = hidden_states.shape[0]
    hidden_states = np.asarray(hidden_states)
    topk_indices = np.asarray(topk_indices)
    shared = prep_shared(
        c, Wq, bq, Wk, bk, Wv, bv, Wo, w_gate, w_up, w_down, ln1_w, ln2_w
    )
    in_maps = []
    for b in range(B):
        for h in range(2):
            in_maps.append(
                prep_core(
                    c,
                    shared,
                    hidden_states[b],
                    topk_indices[b],
                    np.asarray(gating_scores)[b],
                    np.asarray(cos)[b],
                    np.asarray(sin)[b],
                    h,
                )
            )
    nc = _get_nc(c)
    res = _run_spmd_cached(c, nc, in_maps)

    return assemble_output(
        dict(hidden_states=hidden_states, topk_indices=topk_indices), res
    )


# revision 28
# speedup vs baseline: 1.1822x; 1.1822x over previous
"""Trainium2 Bass kernel for the DynamicBlock (ragged top-k decoder layer).

Sharding: 8 cores = (batch b in 0..3) x (query-half h in 0..1).
Core (b, h) processes queries k in [h*512, (h+1)*512) of the K=1024 selected
rows of batch b (causal: needs K/V for all 1024 selected rows, computed
locally -- no collectives).  Untouched hidden rows are assembled host-side.
Matmuls run in bf16 with fp32 accumulation; norms/softmax/residual/gating
in fp32.

Attention exploits causality uniformly across cores: keys are permuted so
this core's own query half is tiles j=0..3 (block-triangular: tile j only
affects query columns >= j*128) and the other half is tiles j=4..7, which
are either fully visible (h=1) or fully masked (h=0) -- expressed as a
per-core additive bias on the exp, so one program serves both core types.
"""

import math
from contextlib import ExitStack
from dataclasses import dataclass

import ml_dtypes
import numpy as np

import concourse.bass as bass
import concourse.mybir as mybir
import concourse.tile as tile
from concourse import bacc
from concourse.bass import IndirectOffsetOnAxis

P = 128
F32 = mybir.dt.float32
BF16 = mybir.dt.bfloat16
F8 = mybir.dt.float8e4
I32 = mybir.dt.int32
AF = mybir.ActivationFunctionType
BF16NP = ml_dtypes.bfloat16
F8NP = ml_dtypes.float8_e4m3
DR = mybir.MatmulPerfMode.DoubleRow


@dataclass(frozen=True)
class Cfg:
    T: int = 4096      # full sequence length
    D: int = 2048      # model dim
    KSEL: int = 1024   # selected rows per sequence
    H: int = 16        # query heads
    KVH: int = 4       # kv heads
    HD: int = 128      # head dim (must equal P)
    FF: int = 8192     # mlp intermediate
    EPS: float = 1e-6

    @property
    def DT(self):
        return self.D // P

    @property
    def QROWS(self):
        return self.KSEL // 2

    @property
    def QT(self):
        return self.QROWS // P

    @property
    def KT(self):
        return self.KSEL // P

    @property
    def FFT(self):
        return self.FF // P

    @property
    def FFG(self):
        return self.FFT // 4

    @property
    def T2(self):
        return self.T // 2

    @property
    def GQ(self):
        return self.H // self.KVH


FULL = Cfg()


def _chunks(total, size):
    out = []
    s = 0
    while s < total:
        out.append((s, min(size, total - s)))
        s += size
    return out


def emit(nc: bass.Bass, c: Cfg, upto: str = "G"):
    _PH = "ABCDEFG"

    def _ph(p):
        return _PH.index(p) <= _PH.index(upto)

    DT, QT, KT, QROWS, KVD = c.DT, c.QT, c.KT, c.QROWS, c.KVH * c.HD
    OGS = _chunks(c.D, 512)  # output-column groups for o-proj / down-proj
    OW = OGS[0][1]

    # ---- DRAM I/O ----
    hid_d = nc.dram_tensor("hid", [c.T, c.D], F32, kind="ExternalInput")
    idxkv_d = nc.dram_tensor("idx_kv", [P, KT], I32, kind="ExternalInput")
    gsc_d = nc.dram_tensor("gsc", [P, QT], F32, kind="ExternalInput")
    cos_d = nc.dram_tensor("cosb", [c.T, c.HD], F32, kind="ExternalInput")
    sin_d = nc.dram_tensor("sinb", [c.T, c.HD], F32, kind="ExternalInput")
    biasj_d = nc.dram_tensor("biasj", [P, KT], F32, kind="ExternalInput")
    tri_d = nc.dram_tensor("tri", [P, P], F8, kind="ExternalInput")
    wq_d = nc.dram_tensor("wq", [c.H, P, DT, c.HD], BF16, kind="ExternalInput")
    wk_d = nc.dram_tensor("wk", [c.KVH, P, DT, c.HD], BF16, kind="ExternalInput")
    wv_d = nc.dram_tensor("wv", [P, DT, KVD], BF16, kind="ExternalInput")
    wo_d = nc.dram_tensor("wo", [len(OGS), c.H // 4, P, 4, OW], BF16, kind="ExternalInput")
    wg_d = nc.dram_tensor("wg", [c.FFG, DT // 4, P, 4, 512], BF16, kind="ExternalInput")
    wu_d = nc.dram_tensor("wu", [c.FFG, DT // 4, P, 4, 512], BF16, kind="ExternalInput")
    wd_d = nc.dram_tensor("wd", [len(OGS), c.FFT // 4, P, 4, OW], BF16, kind="ExternalInput")
    bq_d = nc.dram_tensor("bq", [c.H, P, 1], F32, kind="ExternalInput")
    bk_d = nc.dram_tensor("bk", [c.KVH, P, 1], F32, kind="ExternalInput")
    bv_d = nc.dram_tensor("bv", [1, KVD], F32, kind="ExternalInput")
    idf_d = nc.dram_tensor("id_f", [P, P], F32, kind="ExternalInput")
    idb_d = nc.dram_tensor("id_b", [P, P], BF16, kind="ExternalInput")
    perm_d = nc.dram_tensor("perm", [P, P], BF16, kind="ExternalInput")
    ones_d = nc.dram_tensor("ones16", [P, 2, P], F8, kind="ExternalInput")

    oupd_d = nc.dram_tensor("out_upd", [QROWS, c.D], F32, kind="ExternalOutput")

    scl = 1.0 / math.sqrt(c.HD)

    with ExitStack() as top:
        tc = top.enter_context(tile.TileContext(nc))
        constp = top.enter_context(tc.tile_pool(name="constp", bufs=1, side="left"))
        residp = top.enter_context(tc.tile_pool(name="residp", bufs=1, side="left"))
        psp = top.enter_context(tc.tile_pool(name="psp", bufs=6, space="PSUM"))
        psbp = top.enter_context(tc.tile_pool(name="psbp", bufs=2, space="PSUM"))

        def ps_tile():
            return psp.tile([P, 512], F32, tag="ps", name="ps")

        def psb_tile():
            return psbp.tile([P, P], BF16, tag="psb", name="psb")

        # ---- constants (indices first: they gate the gathers) ----
        idxkv = constp.tile([P, KT], I32, tag="idxkv")
        nc.sync.dma_start(idxkv[:], idxkv_d[:])
        idf = constp.tile([P, P], F32, tag="idf")
        nc.sync.dma_start(idf[:], idf_d[:])
        idb = constp.tile([P, P], BF16, tag="idb")
        nc.sync.dma_start(idb[:], idb_d[:])
        perm = constp.tile([P, P], BF16, tag="perm")
        nc.sync.dma_start(perm[:], perm_d[:])
        ones16 = constp.tile([P, 2, P], F8, tag="ones16")
        nc.sync.dma_start(ones16[:], ones_d[:])
        tri = constp.tile([P, P], F8, tag="tri")
        nc.sync.dma_start(tri[:], tri_d[:])
        gsc = constp.tile([P, QT], F32, tag="gsc")
        nc.sync.dma_start(gsc[:], gsc_d[:])
        biasjc = constp.tile([P, KT], F32, tag="biasjc")
        nc.sync.dma_start(biasjc[:], biasj_d[:])
        bqc = constp.tile([P, c.H], F32, tag="bqc")
        for m in range(c.H):
            nc.sync.dma_start(bqc[:, m : m + 1], bq_d[m])
        bkc = constp.tile([P, c.KVH], F32, tag="bkc")
        for m in range(c.KVH):
            nc.sync.dma_start(bkc[:, m : m + 1], bk_d[m])
        epsc = constp.tile([P, 1], F32, tag="epsc")
        nc.vector.memset(epsc[:], c.EPS)
        bvbc = constp.tile([P, KVD], F32, tag="bvbc")
        bv_ap = bv_d[:]
        nc.sync.dma_start(
            bvbc[:], bass.AP(tensor=bv_ap.tensor, offset=0, ap=[[0, P], [1, KVD]])
        )

        # first-residual (lives B -> E; freed before the MLP phases)
        es_xq = ExitStack()
        xqp = es_xq.enter_context(tc.tile_pool(name="xqp", bufs=1, side="left"))
        xq_raw = xqp.tile([P, QT, c.D], F32, tag="xq_raw")

        sgw = math.gcd(512, c.D)
        nsub = c.D // sgw

        es_bt = ExitStack()  # xkvT/xqT/cos/sin: freed after projections
        xtp = es_bt.enter_context(tc.tile_pool(name="xtp", bufs=1, side="left"))
        xkvT = xtp.tile([P, DT, c.KSEL], BF16, tag="xkvT")
        cosTkv = xtp.tile([P, c.KSEL], F32, tag="cosTkv")
        sinTkv = xtp.tile([P, c.KSEL], F32, tag="sinTkv")
        # host permutes the key order so this core's own query half is rows
        # [0, QROWS) -- q-side tensors are static slices of the kv tensors
        xqT = xkvT[:, :, :QROWS]
        cosTq = cosTkv[:, :QROWS]
        sinTq = sinTkv[:, :QROWS]

        # ---- phase B: gather + rmsnorm1 + transpose ----
        def gather_rows(dst, src_dram, idx_tile, col):
            """Indirect row gather with a gpsimd shield op.

            The shield write/read absorbs the WAR (slot reuse) and RAW (index
            load) waits into a compute op on the triggering engine -- the
            dynamic-queue DMA itself only supports a single sync wait.
            """
            nc.gpsimd.tensor_copy(dst[0:1, 0:1], idx_tile[0:1, col : col + 1])
            nc.gpsimd.indirect_dma_start(
                out=dst,
                out_offset=None,
                in_=src_dram[:],
                in_offset=IndirectOffsetOnAxis(ap=idx_tile[:, col : col + 1], axis=0),
            )

        def norm_transpose(raw, xn_out_fn):
            """raw: [P, D] f32 tile; writes bf16 normalized transposed tiles.

            RMSNorm needs only E[x^2]: one tensor_tensor_reduce (x*x with an
            add-reduction) replaces the bn_stats/bn_aggr chain.
            """
            stats = spool.tile([P, nsub, 6], F32, tag="stats")
            for s in range(nsub):
                nc.vector.bn_stats(stats[:, s, :], raw[:, s * sgw : (s + 1) * sgw])
            mv = spool.tile([P, 2], F32, tag="mv")
            nc.vector.bn_aggr(mv[:], stats[:])
            msq = spool.tile([P, 1], F32, tag="msq")
            nc.vector.tensor_mul(msq[:], mv[:, 0:1], mv[:, 0:1])
            nc.vector.tensor_add(msq[:], msq[:], mv[:, 1:2])
            srt = spool.tile([P, 1], F32, tag="srt")
            nc.scalar.activation(srt[:], msq[:], AF.Sqrt, bias=epsc[:])
            rstd = spool.tile([P, 1], F32, tag="rstd")
            nc.vector.reciprocal(rstd[:], srt[:])
            xn = gpool.tile([P, c.D], BF16, tag="xn")
            nc.vector.tensor_scalar_mul(xn[:], raw[:], rstd[:])
            for dt in range(DT):
                tp = psb_tile()
                nc.tensor.transpose(tp[:], xn[:, dt * P : (dt + 1) * P], idb[:])
                nc.scalar.copy(xn_out_fn(dt), tp[:])

        with tc.tile_pool(name="gpool", bufs=4, side="left") as gpool, tc.tile_pool(name="spool", bufs=4, side="left") as spool, tc.tile_pool(name="cpool", bufs=3, side="left") as cpool:
            for t in range(KT if _ph("B") else 0):
                if t < QT:
                    raw = xq_raw[:, t, :]
                else:
                    raw = gpool.tile([P, c.D], F32, tag="graw", name="graw")[:]
                gather_rows(raw, hid_d, idxkv, t)
                norm_transpose(
                    raw, lambda dt, t=t: xkvT[:, dt, t * P : (t + 1) * P]
                )
            # cos/sin gathers + transposes (f32)
            for srcd, idxt, nt, dst in () if not _ph("B") else (
                (cos_d, idxkv, KT, cosTkv),
                (sin_d, idxkv, KT, sinTkv),
            ):
                for t in range(nt):
                    cg = cpool.tile([P, c.HD], F32, tag="cg")
                    gather_rows(cg[:], srcd, idxt, t)
                    tp = ps_tile()
                    nc.tensor.transpose(tp[:, :P], cg[:], idf[:])
                    nc.scalar.copy(dst[:, t * P : (t + 1) * P], tp[:, :P])

        # ---- phases C+D: projections + rope, interleaved with attention ----
        # Emission order: K proj, V proj, Q(0), Q(1), then per attention head
        # h: Q(h+2) and scores(h+1) are emitted BEFORE AV/ones(h), so the PE
        # queue always has dense independent matmuls to run while the
        # exp->mask chain of the current head completes on scalar/vector.
        es_qkv = ExitStack()
        qkvp = es_qkv.enter_context(tc.tile_pool(name="qkvp", bufs=1, side="right"))
        kT = qkvp.tile([P, c.KVH, c.KSEL], BF16, tag="kT")
        vN = qkvp.tile([P, KT, KVD], BF16, tag="vN")
        qT = qkvp.tile([P, c.H, QROWS], BF16, tag="qT")

        # o-proj weight prefetch: fresh SBUF region + early queue position so
        # chunks stream in during C/D and phase E never waits on weights
        es_wo = ExitStack()
        wop = es_wo.enter_context(tc.tile_pool(name="wop", bufs=4, side="left"))
        wo_tiles = {}
        for ogi in range(len(OGS) if _ph("E") else 0):
            for c4 in range(c.H // 4):
                wt = wop.tile([P, 4, OW], BF16, tag="wot")
                # gpsimd-engine queue: decoupled from the sync queue so these
                # prefetches never head-of-line-block the Q/K weight streams
                nc.gpsimd.dma_start(wt[:], wo_d[ogi, c4])
                wo_tiles[(ogi, c4)] = wt

        es_attn = ExitStack()
        attnp = es_attn.enter_context(tc.tile_pool(name="attnp", bufs=1, side="left"))
        xattnT = attnp.tile([P, c.H, QROWS], BF16, tag="xattnT")

        # Causal structure (keys permuted: own half first):
        #   tile j<4: affects only query cols >= j*128; diagonal 128x128
        #     sub-block is triangular (tri multiply); rest fully visible.
        #   tile j>=4: all-visible (h=1) or all-masked (h=0) -- via biasj.
        # exp computes exp(scl*s + biasj) with biasj in {-1, -30001}: the
        # uniform -1 shift guards fp8 overflow and cancels in the divide.
        # expT is fp8 (softmax weights in [0, e^4.5]); the softmax-sum uses a
        # DoubleRow fp8 matmul against a constant 16.0 stationary (the x16
        # is divided back out in the final normalization).
        def jow(j):
            o = j * P if j < 4 else 0
            return o, QROWS - o

        with tc.tile_pool(name="wstr", bufs=3, side="left") as wstr, tc.tile_pool(name="rpool", bufs=3, side="left") as rpool, tc.tile_pool(name="ropep", bufs=2, side="left") as ropep, tc.tile_pool(name="dpool", bufs=3, side="left") as dpool, tc.tile_pool(name="recp", bufs=3, side="left") as recp:

            def rope(dst, rawt, rot_ps, cosT, sinT, s0, w):
                t1 = ropep.tile([P, 512], F32, tag="ropet1")
                nc.vector.tensor_mul(t1[:, :w], rawt[:, s0 : s0 + w], cosT[:, s0 : s0 + w])
                t2 = ropep.tile([P, 512], F32, tag="ropet2")
                nc.vector.tensor_mul(t2[:, :w], rot_ps[:, :w], sinT[:, s0 : s0 + w])
                nc.vector.tensor_add(dst[:, s0 : s0 + w], t1[:, :w], t2[:, :w])

            def qproj(m):
                wqm = wstr.tile([P, DT, c.HD], BF16, tag="wqkm", name="wqm")
                nc.sync.dma_start(wqm[:], wq_d[m])
                qraw = rpool.tile([P, c.KSEL], BF16, tag="kqraw", name="qraw")
                ps = ps_tile()
                for dt in range(DT):
                    nc.tensor.matmul(
                        ps[:, :QROWS],
                        wqm[:, dt, :],
                        xqT[:, dt, :],
                        start=(dt == 0),
                        stop=(dt == DT - 1),
                    )
                nc.vector.tensor_scalar_add(
                    qraw[:, :QROWS], ps[:, :QROWS], bqc[:, m : m + 1]
                )
                rot = ps_tile()
                nc.tensor.matmul(
                    rot[:, :QROWS], perm[:], qraw[:, :QROWS], start=True, stop=True
                )
                rope(qT[:, m, :], qraw, rot, cosTq, sinTq, 0, QROWS)

            def kproj(m):
                wkm = wstr.tile([P, DT, c.HD], BF16, tag="wqkm", name="wkm")
                nc.sync.dma_start(wkm[:], wk_d[m])
                kraw = rpool.tile([P, c.KSEL], BF16, tag="kqraw")
                for s0, w in _chunks(c.KSEL, 512):
                    ps = ps_tile()
                    for dt in range(DT):
                        nc.tensor.matmul(
                            ps[:, :w],
                            wkm[:, dt, :],
                            xkvT[:, dt, s0 : s0 + w],
                            start=(dt == 0),
                            stop=(dt == DT - 1),
                        )
                    nc.vector.tensor_scalar_add(
                        kraw[:, s0 : s0 + w], ps[:, :w], bkc[:, m : m + 1]
                    )
                for s0, w in _chunks(c.KSEL, 512):
                    rot = ps_tile()
                    nc.tensor.matmul(
                        rot[:, :w], perm[:], kraw[:, s0 : s0 + w], start=True, stop=True
                    )
                    rope(kT[:, m, :], kraw, rot, cosTkv, sinTkv, s0, w)

            exp_tiles = {}

            def scores_block(h):
                g = h // c.GQ
                expT = dpool.tile([P, KT, QROWS], F8, tag="expT")
                exp_tiles[h] = expT
                # zero the fp8 pair-mate gaps (cols outside a tile's causal
                # range that its DoubleRow partner still streams)
                nc.gpsimd.memset(expT[:, 1, 0:P], 0.0)
                nc.gpsimd.memset(expT[:, 3, 2 * P : 3 * P], 0.0)
                for j in range(KT):
                    o, w = jow(j)
                    ps = ps_tile()
                    nc.tensor.matmul(
                        ps[:, o:QROWS],
                        kT[:, g, j * P : (j + 1) * P],
                        qT[:, h, o:QROWS],
                        start=True,
                        stop=True,
                    )
                    nc.scalar.activation(
                        expT[:, j, o:QROWS], ps[:, o:QROWS], AF.Exp,
                        scale=scl, bias=biasjc[:, j : j + 1],
                    )
                    if j < 4:
                        nc.vector.tensor_mul(
                            expT[:, j, j * P : (j + 1) * P],
                            expT[:, j, j * P : (j + 1) * P],
                            tri[:],
                        )

            def av_ones_drain(h):
                g = h // c.GQ
                expT = exp_tiles.pop(h)
                pso = ps_tile()
                pss = ps_tile()
                for j in range(KT):
                    o, w = jow(j)
                    nc.tensor.matmul(
                        pso[:, o:QROWS],
                        vN[:, j, g * c.HD : (g + 1) * c.HD],
                        expT[:, j, o:QROWS],
                        start=(j == 0),
                        stop=(j == KT - 1),
                        skip_group_check=True,
                    )
                for p2 in range(KT // 2):
                    o, w = jow(2 * p2)
                    nc.tensor.matmul(
                        pss[:, o:QROWS],
                        ones16[:],
                        expT[:, 2 * p2 : 2 * p2 + 2, o:QROWS],
                        start=(p2 == 0),
                        stop=(p2 == KT // 2 - 1),
                        skip_group_check=True,
                        perf_mode=DR,
                    )
                rec = recp.tile([P, QROWS], F32, tag="rec")
                nc.vector.reciprocal_approx_fast(rec[:], pss[:, :QROWS])
                # pss = 16*sum(exp): fold the /16 back via the 16.0 scalar
                nc.vector.scalar_tensor_tensor(
                    xattnT[:, h, :], pso[:, :QROWS], 16.0, rec[:],
                    mybir.AluOpType.mult, mybir.AluOpType.mult,
                )

            # K + V first, then Q, then attention (scores one head ahead)
            with tc.tile_pool(name="wvp", bufs=1, side="left") as wvp:
                wvsb = wvp.tile([P, DT, KVD], BF16, tag="wvsb")
                nc.sync.dma_start(wvsb[:], wv_d[:])
                for m in range(c.KVH if _ph("C") else 0):
                    kproj(m)
                for rt in range(KT if _ph("C") else 0):
                    psv = ps_tile()
                    for dt in range(DT):
                        nc.tensor.matmul(
                            psv[:, :KVD],
                            xkvT[:, dt, rt * P : (rt + 1) * P],
                            wvsb[:, dt, :],
                            start=(dt == 0),
                            stop=(dt == DT - 1),
                        )
                    nc.vector.tensor_add(vN[:, rt, :], psv[:, :KVD], bvbc[:])

            for m in range(c.H if _ph("C") else 0):
                qproj(m)

            if _ph("D"):
                scores_block(0)
                for h in range(c.H):
                    if h + 1 < c.H:
                        scores_block(h + 1)
                    av_ones_drain(h)

        es_qkv.close()  # free kT/vN/qT (right side)

        # attn+mlp residual, lives E -> G
        es_res2 = ExitStack()
        res2p = es_res2.enter_context(tc.tile_pool(name="res2p", bufs=1, side="right"))
        res2 = res2p.tile([P, QT, c.D], F32, tag="res2")

        # ---- phase E: o-proj + residual + rmsnorm2 ----
        es_xm = ExitStack()
        xmp = es_xm.enter_context(tc.tile_pool(name="xmp", bufs=1, side="right"))
        xmT = xmp.tile([P, DT, QROWS], BF16, tag="xmT")

        with tc.tile_pool(name="gpool2", bufs=3, side="left") as gpool2, tc.tile_pool(name="spool2", bufs=4, side="left") as spool2:
            def _norm2_transpose(qt):
                mv = spool2.tile([P, 2], F32, tag="mv2", name="mv")
                nc.vector.bn_aggr(mv[:], stats2[:, qt])
                msq = spool2.tile([P, 1], F32, tag="msq2", name="msq")
                nc.vector.tensor_mul(msq[:], mv[:, 0:1], mv[:, 0:1])
                nc.vector.tensor_add(msq[:], msq[:], mv[:, 1:2])
                srt = spool2.tile([P, 1], F32, tag="srt2", name="srt")
                nc.scalar.activation(srt[:], msq[:], AF.Sqrt, bias=epsc[:])
                rstd = spool2.tile([P, 1], F32, tag="rstd2", name="rstd")
                nc.vector.reciprocal(rstd[:], srt[:])
                xn = gpool2.tile([P, c.D], BF16, tag="xn2", name="xn")
                nc.vector.tensor_scalar_mul(xn[:], res2[:, qt, :], rstd[:])
                for dt in range(DT):
                    tp = psb_tile()
                    nc.tensor.transpose(
                        tp[:], xn[:, dt * P : (dt + 1) * P], idb[:]
                    )
                    nc.scalar.copy(xmT[:, dt, qt * P : (qt + 1) * P], tp[:])

            stats2 = spool2.tile([P, QT, nsub, 6], F32, tag="stats2all")
            for ogi, (os_, ow) in enumerate(OGS if _ph("E") else []):
                pss4 = [ps_tile() for _ in range(QT)]
                for c4 in range(c.H // 4):
                    wot = wo_tiles[(ogi, c4)]
                    for i in range(4):
                        ht = c4 * 4 + i
                        for qt in range(QT):
                            nc.tensor.matmul(
                                pss4[qt][:, :ow],
                                xattnT[:, ht, qt * P : (qt + 1) * P],
                                wot[:, i, :ow],
                                start=(ht == 0),
                                stop=(ht == c.H - 1),
                            )
                for qt in range(QT):
                    nc.vector.tensor_add(
                        res2[:, qt, os_ : os_ + ow],
                        pss4[qt][:, :ow],
                        xq_raw[:, qt, os_ : os_ + ow],
                    )
                    # incremental norm2 stats: OGS chunks == bn subgroups
                    nc.vector.bn_stats(
                        stats2[:, qt, ogi, :], res2[:, qt, os_ : os_ + ow]
                    )
            for qt in range(QT if _ph("E") else 0):
                _norm2_transpose(qt)
            for qt in range(QT if _ph("E") else 0):
                # fold gating: res2 := (res2 - xq_raw)*g + xq_raw, so phase G
                # only needs one fused op per output tile after the last MM
                nc.vector.tensor_sub(
                    res2[:, qt, :], res2[:, qt, :], xq_raw[:, qt, :]
                )
                nc.vector.tensor_scalar_mul(
                    res2[:, qt, :], res2[:, qt, :], gsc[:, qt : qt + 1]
                )
                nc.vector.tensor_add(
                    res2[:, qt, :], res2[:, qt, :], xq_raw[:, qt, :]
                )

        es_attn.close()  # free xattnT
        es_wo.close()    # free wo tiles
        es_bt.close()    # free xkvT/xqT/cos/sin (held through D for Q-proj)
        es_xq.close()    # free xq_raw (gating already folded into res2)

        # ---- phase F: mlp gate/up ----
        es_act = ExitStack()
        actp = es_act.enter_context(tc.tile_pool(name="actp", bufs=1, side="left"))
        actT = actp.tile([P, c.FFT, QROWS], BF16, tag="actT")

        es_wd = ExitStack()  # down-proj weight stream: spans F (prefetch) + G
        wstr4 = es_wd.enter_context(tc.tile_pool(name="wstr4", bufs=6, side="left"))
        wd_pre = []
        for pi in range(2 if _ph("G") else 0):
            wdt = wstr4.tile([P, 4, OW], BF16, tag="wdt")
            nc.gpsimd.dma_start(wdt[:], wd_d[0, pi])
            wd_pre.append(wdt)

        with tc.tile_pool(name="wstr3", bufs=6, side="left") as wstr3, tc.tile_pool(name="fpool", bufs=3, side="left") as fpool:
            for g in range(c.FFG if _ph("F") else 0):
                psg = [ps_tile() for _ in range(4)]
                for d4 in range(DT // 4):
                    wgt = wstr3.tile([P, 4, 512], BF16, tag="wgut")
                    nc.sync.dma_start(wgt[:], wg_d[g, d4])
                    for i in range(4):
                        dt = d4 * 4 + i
                        for s in range(4):
                            nc.tensor.matmul(
                                psg[s][:, :QROWS],
                                wgt[:, i, s * P : (s + 1) * P],
                                xmT[:, dt, :],
                                start=(dt == 0),
                                stop=(dt == DT - 1),
                            )
                silu = fpool.tile([P, 4, QROWS], F32, tag="silu")
                for s in range(4):
                    # silu(x) = x * sigmoid(x) (Silu isn't in CoreSim)
                    nc.scalar.activation(silu[:, s, :], psg[s][:, :QROWS], AF.Sigmoid)
                    nc.vector.tensor_mul(silu[:, s, :], silu[:, s, :], psg[s][:, :QROWS])
                psu = [ps_tile() for _ in range(4)]
                for d4 in range(DT // 4):
                    wut = wstr3.tile([P, 4, 512], BF16, tag="wgut")
                    nc.sync.dma_start(wut[:], wu_d[g, d4])
                    for i in range(4):
                        dt = d4 * 4 + i
                        for s in range(4):
                            nc.tensor.matmul(
                                psu[s][:, :QROWS],
                                wut[:, i, s * P : (s + 1) * P],
                                xmT[:, dt, :],
                                start=(dt == 0),
                                stop=(dt == DT - 1),
                            )
                for s in range(4):
                    nc.vector.tensor_mul(
                        actT[:, g * 4 + s, :], silu[:, s, :], psu[s][:, :QROWS]
                    )

        es_xm.close()  # free xmT

        # ---- phase G: down-proj + residual + gating + output ----
        with tc.tile_pool(name="opool", bufs=3, side="left") as opool:
            for ogi, (os_, ow) in enumerate(OGS if _ph("G") else []):
                psd = [ps_tile() for _ in range(QT)]
                for f4 in range(c.FFT // 4):
                    if ogi == 0 and f4 < 2:
                        wdt = wd_pre[f4]
                    else:
                        wdt = wstr4.tile([P, 4, OW], BF16, tag="wdt")
                        nc.sync.dma_start(wdt[:], wd_d[ogi, f4])
                    for i in range(4):
                        ffp = f4 * 4 + i
                        for qt in range(QT):
                            nc.tensor.matmul(
                                psd[qt][:, :ow],
                                actT[:, ffp, qt * P : (qt + 1) * P],
                                wdt[:, i, :ow],
                                start=(ffp == 0),
                                stop=(ffp == c.FFT - 1),
                            )
                for qt in range(QT):
                    t1 = opool.tile([P, 512], F32, tag="updt")
                    nc.vector.scalar_tensor_tensor(
                        t1[:, :ow],
                        psd[qt][:, :ow],
                        gsc[:, qt : qt + 1],
                        res2[:, qt, os_ : os_ + ow],
                        mybir.AluOpType.mult,
                        mybir.AluOpType.add,
                    )
                    nc.sync.dma_start(
                        oupd_d[qt * P : (qt + 1) * P, os_ : os_ + ow], t1[:, :ow]
                    )

        es_wd.close()
        es_act.close()
        es_res2.close()
    return nc


# ---------------- host side ----------------


def _bf(x):
    return np.ascontiguousarray(x.astype(BF16NP))


def _f32(x):
    return np.ascontiguousarray(x, dtype=np.float32)


def prep_shared(c: Cfg, Wq, bq, Wk, bk, Wv, bv, Wo, w_gate, w_up, w_down, ln1_w, ln2_w):
    """Host-side weight folding + tiling (exact fp32 math, then bf16 cast)."""
    DT, FFT, FFG, KVD = c.DT, c.FFT, c.FFG, c.KVH * c.HD
    OGS = _chunks(c.D, 512)
    OG, OW = len(OGS), OGS[0][1]
    Wqf = _f32(Wq) * _f32(ln1_w)[:, None]
    Wkf = _f32(Wk) * _f32(ln1_w)[:, None]
    Wvf = _f32(Wv) * _f32(ln1_w)[:, None]
    Wgf = _f32(w_gate) * _f32(ln2_w)[:, None]
    Wuf = _f32(w_up) * _f32(ln2_w)[:, None]

    perm = np.zeros((P, P), np.float32)
    half = c.HD // 2
    perm[np.arange(half) + half, np.arange(half)] = -1.0
    perm[np.arange(half), np.arange(half) + half] = 1.0

    # tri[k, q] = 1 if k <= q (keep) else 0, for the diagonal 128x128 block
    tri = np.triu(np.ones((P, P), np.float32))

    return dict(
        wq=_bf(Wqf.reshape(DT, P, c.H, c.HD).transpose(2, 1, 0, 3)),
        wk=_bf(Wkf.reshape(DT, P, c.KVH, c.HD).transpose(2, 1, 0, 3)),
        wv=_bf(Wvf.reshape(DT, P, KVD).transpose(1, 0, 2)),
        # wo[ogi, c4, p, i, col] = Wo[(c4*4+i)*128+p, ogi*512+col]
        wo=_bf(_f32(Wo).reshape(c.H // 4, 4, P, OG, OW).transpose(3, 0, 2, 1, 4)),
        # wg[g, d4, p, i, col] = Wgf[(d4*4+i)*128+p, g*512+col]
        wg=_bf(Wgf.reshape(DT // 4, 4, P, FFG, 512).transpose(3, 0, 2, 1, 4)),
        wu=_bf(Wuf.reshape(DT // 4, 4, P, FFG, 512).transpose(3, 0, 2, 1, 4)),
        # wd[ogi, f4, p, i, col] = w_down[(f4*4+i)*128+p, ogi*512+col]
        wd=_bf(_f32(w_down).reshape(FFT // 4, 4, P, OG, OW).transpose(3, 0, 2, 1, 4)),
        bq=_f32(bq).reshape(c.H, P, 1),
        bk=_f32(bk).reshape(c.KVH, P, 1),
        bv=_f32(bv).reshape(1, KVD),
        id_f=np.eye(P, dtype=np.float32),
        id_b=np.eye(P, dtype=np.float32).astype(BF16NP),
        perm=perm.astype(BF16NP),
        ones16=np.full((P, 2, P), 16.0, np.float32).astype(F8NP),
        tri=tri.astype(F8NP),
    )


def prep_core(c: Cfg, shared, hid_b, idx_b, g_b, cos_b, sin_b, h):
    """Per-core inputs for core handling query-half h of one batch."""
    QROWS, QT, KT = c.QROWS, c.QT, c.KT
    idx32 = idx_b.astype(np.int32)
    # permute keys so this core's own query half comes first; block-causal
    # masking is then uniform: tile j<4 affects only cols >= j*128 with a
    # triangular diagonal block; tiles j>=4 are all-or-nothing via biasj
    kperm = np.concatenate(
        [np.arange(h * QROWS, (h + 1) * QROWS),
         np.arange(0, h * QROWS), np.arange((h + 1) * QROWS, c.KSEL)]
    )
    idx32 = idx32[kperm]
    # exp bias per key tile: -1 everywhere (overflow guard, cancels in the
    # normalization); other-half tiles fully masked for h=0 cores
    biasj = np.full((P, KT), -1.0, np.float32)
    if h == 0:
        biasj[:, 4:] = -30001.0
    m = dict(
        hid=_f32(hid_b),
        idx_kv=np.ascontiguousarray(idx32.reshape(KT, P).T),
        gsc=np.ascontiguousarray(
            _f32(g_b[h * QROWS : (h + 1) * QROWS]).reshape(QT, P).T
        ),
        cosb=_f32(cos_b),
        sinb=_f32(sin_b),
        biasj=biasj,
    )
    m.update(shared)
    return m


_NC_CACHE = {}


def _get_nc(c: Cfg):
    key = c
    if key not in _NC_CACHE:
        nc = bacc.Bacc()
        emit(nc, c)
        nc.compile()
        _NC_CACHE[key] = nc
    return _NC_CACHE[key]


_RUN_CACHE = {}


def _run_spmd_cached(c: Cfg, nc, in_maps):
    """run_bass_via_pjrt equivalent with a cached jitted executable.

    run_bass_kernel_spmd rebuilds its jit closure per call, so every kernel()
    invocation would re-trace + recompile (~40s).  Build the shard_map jit
    once per config and reuse it; repeat calls only pay host->device
    transfer + execution.
    """
    import jax
    import numpy as np
    from jax.sharding import Mesh, PartitionSpec
    from jax.experimental.shard_map import shard_map
    from concourse import bass2jax
    from concourse.bass2jax import _bass_exec_p, install_neuronx_cc_hook

    n_cores = len(in_maps)
    key = (c, n_cores)
    if key not in _RUN_CACHE:
        install_neuronx_cc_hook()
        partition_name = (
            nc.partition_id_tensor.name if nc.partition_id_tensor else None
        )
        in_names, out_names, out_avals = [], [], []
        for alloc in nc.m.functions[0].allocations:
            if not isinstance(alloc, mybir.MemoryLocationSet):
                continue
            name = alloc.memorylocations[0].name
            if alloc.kind == "ExternalInput":
                if name != partition_name:
                    in_names.append(name)
            elif alloc.kind == "ExternalOutput":
                out_names.append(name)
                out_avals.append(
                    jax.core.ShapedArray(
                        tuple(alloc.tensor_shape), mybir.dt.np(alloc.dtype)
                    )
                )
        n_params = len(in_names)
        all_in = list(in_names) + list(out_names)
        if partition_name is not None:
            all_in.append(partition_name)

        def _body(*flat):
            operands = list(flat)
            if partition_name is not None:
                operands.append(bass2jax.partition_id_tensor())
            return tuple(
                _bass_exec_p.bind(
                    *operands,
                    out_avals=tuple(out_avals),
                    in_names=tuple(all_in),
                    out_names=tuple(out_names),
                    lowering_input_output_aliases=(),
                    sim_require_finite=True,
                    sim_require_nnan=True,
                    nc=nc,
                )
            )

        devices = jax.devices()[:n_cores]
        mesh = Mesh(np.asarray(devices), ("core",))
        n_outs = len(out_avals)
        sharded = jax.jit(
            shard_map(
                _body,
                mesh=mesh,
                in_specs=(PartitionSpec("core"),) * (n_params + n_outs),
                out_specs=(PartitionSpec("core"),) * n_outs,
                check_rep=False,
            ),
            keep_unused=True,
        )
        zeros = [
            np.zeros((n_cores * a.shape[0], *a.shape[1:]), a.dtype)
            for a in out_avals
        ]
        _RUN_CACHE[key] = (sharded, in_names, out_names, out_avals, zeros)

    sharded, in_names, out_names, out_avals, zeros = _RUN_CACHE[key]
    concat_in = [
        np.concatenate([np.asarray(in_maps[ci][nm]) for ci in range(n_cores)], axis=0)
        for nm in in_names
    ]
    out_arrs = sharded(*concat_in, *zeros)
    return [
        {
            name: np.asarray(out_arrs[i]).reshape(n_cores, *out_avals[i].shape)[ci]
            for i, name in enumerate(out_names)
        }
        for ci in range(n_cores)
    ]


def assemble_output(inputs, res):
    """Build the full [B, T, D] output from per-core result maps."""
    c = FULL
    hidden_states = np.asarray(inputs["hidden_states"])
    topk = np.asarray(inputs["topk_indices"])
    B = hidden_states.shape[0]
    final = np.ascontiguousarray(hidden_states, dtype=np.float32).copy()
    for ci in range(2 * B):
        b, h = ci // 2, ci % 2
        sel = topk[b, h * c.QROWS : (h + 1) * c.QROWS].astype(np.int64)
        final[b, sel] = res[ci]["out_upd"]
    return final


def kernel(
    hidden_states,
    topk_indices,
    gating_scores,
    cos,
    sin,
    Wq,
    bq,
    Wk,
    bk,
    Wv,
    bv,
    Wo,
    w_gate,
    w_up,
    w_down,
    ln1_w,
    ln2_w,
):
    c = FULL
    B = hidden_states.shape[0]
    hidden_states = np.asarray(hidden_states)
    topk_indices = np.asarray(topk_indices)
    shared = prep_shared(
        c, Wq, bq, Wk, bk, Wv, bv, Wo, w_gate, w_up, w_down, ln1_w, ln2_w
    )
    in_maps = []
    for b in range(B):
        for h in range(2):
            in_maps.append(
                prep_core(
                    c,
                    shared,
                    hidden_states[b],
                    topk_indices[b],
                    np.asarray(gating_scores)[b],
                    np.asarray(cos)[b],
                    np.asarray(sin)[b],
                    h,
                )
            )
    nc = _get_nc(c)
    res = _run_spmd_cached(c, nc, in_maps)

    return assemble_output(
        dict(hidden_states=hidden_states, topk_indices=topk_indices), res
    )


# revision 29
# speedup vs baseline: 1.1847x; 1.0021x over previous
"""Trainium2 Bass kernel for the DynamicBlock (ragged top-k decoder layer).

Sharding: 8 cores = (batch b in 0..3) x (query-half h in 0..1).
Core (b, h) processes queries k in [h*512, (h+1)*512) of the K=1024 selected
rows of batch b (causal: needs K/V for all 1024 selected rows, computed
locally -- no collectives).  Untouched hidden rows are assembled host-side.
Matmuls run in bf16 with fp32 accumulation; norms/softmax/residual/gating
in fp32.

Attention exploits causality uniformly across cores: keys are permuted so
this core's own query half is tiles j=0..3 (block-triangular: tile j only
affects query columns >= j*128) and the other half is tiles j=4..7, which
are either fully visible (h=1) or fully masked (h=0) -- expressed as a
per-core additive bias on the exp, so one program serves both core types.
"""

import math
from contextlib import ExitStack
from dataclasses import dataclass

import ml_dtypes
import numpy as np

import concourse.bass as bass
import concourse.mybir as mybir
import concourse.tile as tile
from concourse import bacc
from concourse.bass import IndirectOffsetOnAxis

P = 128
F32 = mybir.dt.float32
BF16 = mybir.dt.bfloat16
F8 = mybir.dt.float8e4
I32 = mybir.dt.int32
AF = mybir.ActivationFunctionType
BF16NP = ml_dtypes.bfloat16
F8NP = ml_dtypes.float8_e4m3
DR = mybir.MatmulPerfMode.DoubleRow


@dataclass(frozen=True)
class Cfg:
    T: int = 4096      # full sequence length
    D: int = 2048      # model dim
    KSEL: int = 1024   # selected rows per sequence
    H: int = 16        # query heads
    KVH: int = 4       # kv heads
    HD: int = 128      # head dim (must equal P)
    FF: int = 8192     # mlp intermediate
    EPS: float = 1e-6

    @property
    def DT(self):
        return self.D // P

    @property
    def QROWS(self):
        return self.KSEL // 2

    @property
    def QT(self):
        return self.QROWS // P

    @property
    def KT(self):
        return self.KSEL // P

    @property
    def FFT(self):
        return self.FF // P

    @property
    def FFG(self):
        return self.FFT // 4

    @property
    def T2(self):
        return self.T // 2

    @property
    def GQ(self):
        return self.H // self.KVH


FULL = Cfg()


def _chunks(total, size):
    out = []
    s = 0
    while s < total:
        out.append((s, min(size, total - s)))
        s += size
    return out


def emit(nc: bass.Bass, c: Cfg, upto: str = "G"):
    _PH = "ABCDEFG"

    def _ph(p):
        return _PH.index(p) <= _PH.index(upto)

    DT, QT, KT, QROWS, KVD = c.DT, c.QT, c.KT, c.QROWS, c.KVH * c.HD
    OGS = _chunks(c.D, 512)  # output-column groups for o-proj / down-proj
    OW = OGS[0][1]

    # ---- DRAM I/O ----
    hid_d = nc.dram_tensor("hid", [c.T, c.D], F32, kind="ExternalInput")
    idxkv_d = nc.dram_tensor("idx_kv", [P, KT], I32, kind="ExternalInput")
    gsc_d = nc.dram_tensor("gsc", [P, QT], F32, kind="ExternalInput")
    cos_d = nc.dram_tensor("cosb", [c.T, c.HD], F32, kind="ExternalInput")
    sin_d = nc.dram_tensor("sinb", [c.T, c.HD], F32, kind="ExternalInput")
    biasj_d = nc.dram_tensor("biasj", [P, KT], F32, kind="ExternalInput")
    tri_d = nc.dram_tensor("tri", [P, P], F8, kind="ExternalInput")
    wq_d = nc.dram_tensor("wq", [c.H, P, DT, c.HD], BF16, kind="ExternalInput")
    wk_d = nc.dram_tensor("wk", [c.KVH, P, DT, c.HD], BF16, kind="ExternalInput")
    wv_d = nc.dram_tensor("wv", [P, DT, KVD], BF16, kind="ExternalInput")
    wo_d = nc.dram_tensor("wo", [len(OGS), c.H // 4, P, 4, OW], BF16, kind="ExternalInput")
    wg_d = nc.dram_tensor("wg", [c.FFG, DT // 4, P, 4, 512], BF16, kind="ExternalInput")
    wu_d = nc.dram_tensor("wu", [c.FFG, DT // 4, P, 4, 512], BF16, kind="ExternalInput")
    wd_d = nc.dram_tensor("wd", [len(OGS), c.FFT // 4, P, 4, OW], BF16, kind="ExternalInput")
    bq_d = nc.dram_tensor("bq", [c.H, P, 1], F32, kind="ExternalInput")
    bk_d = nc.dram_tensor("bk", [c.KVH, P, 1], F32, kind="ExternalInput")
    bv_d = nc.dram_tensor("bv", [1, KVD], F32, kind="ExternalInput")
    idf_d = nc.dram_tensor("id_f", [P, P], F32, kind="ExternalInput")
    idb_d = nc.dram_tensor("id_b", [P, P], BF16, kind="ExternalInput")
    perm_d = nc.dram_tensor("perm", [P, P], BF16, kind="ExternalInput")
    ones_d = nc.dram_tensor("ones16", [P, 2, P], F8, kind="ExternalInput")

    oupd_d = nc.dram_tensor("out_upd", [QROWS, c.D], F32, kind="ExternalOutput")

    scl = 1.0 / math.sqrt(c.HD)

    with ExitStack() as top:
        tc = top.enter_context(tile.TileContext(nc))
        constp = top.enter_context(tc.tile_pool(name="constp", bufs=1, side="left"))
        residp = top.enter_context(tc.tile_pool(name="residp", bufs=1, side="left"))
        psp = top.enter_context(tc.tile_pool(name="psp", bufs=6, space="PSUM"))
        psbp = top.enter_context(tc.tile_pool(name="psbp", bufs=2, space="PSUM"))

        def ps_tile():
            return psp.tile([P, 512], F32, tag="ps", name="ps")

        def psb_tile():
            return psbp.tile([P, P], BF16, tag="psb", name="psb")

        # ---- constants (indices first: they gate the gathers) ----
        idxkv = constp.tile([P, KT], I32, tag="idxkv")
        nc.sync.dma_start(idxkv[:], idxkv_d[:])
        idf = constp.tile([P, P], F32, tag="idf")
        nc.sync.dma_start(idf[:], idf_d[:])
        idb = constp.tile([P, P], BF16, tag="idb")
        nc.sync.dma_start(idb[:], idb_d[:])
        perm = constp.tile([P, P], BF16, tag="perm")
        nc.sync.dma_start(perm[:], perm_d[:])
        ones16 = constp.tile([P, 2, P], F8, tag="ones16")
        nc.sync.dma_start(ones16[:], ones_d[:])
        tri = constp.tile([P, P], F8, tag="tri")
        nc.sync.dma_start(tri[:], tri_d[:])
        gsc = constp.tile([P, QT], F32, tag="gsc")
        nc.sync.dma_start(gsc[:], gsc_d[:])
        biasjc = constp.tile([P, KT], F32, tag="biasjc")
        nc.sync.dma_start(biasjc[:], biasj_d[:])
        bqc = constp.tile([P, c.H], F32, tag="bqc")
        for m in range(c.H):
            nc.sync.dma_start(bqc[:, m : m + 1], bq_d[m])
        bkc = constp.tile([P, c.KVH], F32, tag="bkc")
        for m in range(c.KVH):
            nc.sync.dma_start(bkc[:, m : m + 1], bk_d[m])
        epsc = constp.tile([P, 1], F32, tag="epsc")
        nc.vector.memset(epsc[:], c.EPS)
        bvbc = constp.tile([P, KVD], F32, tag="bvbc")
        bv_ap = bv_d[:]
        nc.sync.dma_start(
            bvbc[:], bass.AP(tensor=bv_ap.tensor, offset=0, ap=[[0, P], [1, KVD]])
        )

        # first-residual (lives B -> E; freed before the MLP phases)
        es_xq = ExitStack()
        xqp = es_xq.enter_context(tc.tile_pool(name="xqp", bufs=1, side="left"))
        xq_raw = xqp.tile([P, QT, c.D], F32, tag="xq_raw")

        sgw = math.gcd(512, c.D)
        nsub = c.D // sgw

        es_bt = ExitStack()  # xkvT/xqT/cos/sin: freed after projections
        xtp = es_bt.enter_context(tc.tile_pool(name="xtp", bufs=1, side="left"))
        xkvT = xtp.tile([P, DT, c.KSEL], BF16, tag="xkvT")
        cosTkv = xtp.tile([P, c.KSEL], F32, tag="cosTkv")
        sinTkv = xtp.tile([P, c.KSEL], F32, tag="sinTkv")
        # host permutes the key order so this core's own query half is rows
        # [0, QROWS) -- q-side tensors are static slices of the kv tensors
        xqT = xkvT[:, :, :QROWS]
        cosTq = cosTkv[:, :QROWS]
        sinTq = sinTkv[:, :QROWS]

        # ---- phase B: gather + rmsnorm1 + transpose ----
        def gather_rows(dst, src_dram, idx_tile, col):
            """Indirect row gather with a gpsimd shield op.

            The shield write/read absorbs the WAR (slot reuse) and RAW (index
            load) waits into a compute op on the triggering engine -- the
            dynamic-queue DMA itself only supports a single sync wait.
            """
            nc.gpsimd.tensor_copy(dst[0:1, 0:1], idx_tile[0:1, col : col + 1])
            nc.gpsimd.indirect_dma_start(
                out=dst,
                out_offset=None,
                in_=src_dram[:],
                in_offset=IndirectOffsetOnAxis(ap=idx_tile[:, col : col + 1], axis=0),
            )

        def norm_transpose(raw, xn_out_fn):
            """raw: [P, D] f32 tile; writes bf16 normalized transposed tiles.

            RMSNorm needs only E[x^2]: one tensor_tensor_reduce (x*x with an
            add-reduction) replaces the bn_stats/bn_aggr chain.
            """
            stats = spool.tile([P, nsub, 6], F32, tag="stats")
            for s in range(nsub):
                nc.vector.bn_stats(stats[:, s, :], raw[:, s * sgw : (s + 1) * sgw])
            mv = spool.tile([P, 2], F32, tag="mv")
            nc.vector.bn_aggr(mv[:], stats[:])
            msq = spool.tile([P, 1], F32, tag="msq")
            nc.vector.tensor_mul(msq[:], mv[:, 0:1], mv[:, 0:1])
            nc.vector.tensor_add(msq[:], msq[:], mv[:, 1:2])
            srt = spool.tile([P, 1], F32, tag="srt")
            nc.scalar.activation(srt[:], msq[:], AF.Sqrt, bias=epsc[:])
            rstd = spool.tile([P, 1], F32, tag="rstd")
            nc.vector.reciprocal(rstd[:], srt[:])
            xn = gpool.tile([P, c.D], BF16, tag="xn")
            nc.vector.tensor_scalar_mul(xn[:], raw[:], rstd[:])
            for dt in range(DT):
                tp = psb_tile()
                nc.tensor.transpose(tp[:], xn[:, dt * P : (dt + 1) * P], idb[:])
                nc.scalar.copy(xn_out_fn(dt), tp[:])

        with tc.tile_pool(name="gpool", bufs=4, side="left") as gpool, tc.tile_pool(name="spool", bufs=4, side="left") as spool, tc.tile_pool(name="cpool", bufs=3, side="left") as cpool:
            for t in range(KT if _ph("B") else 0):
                if t < QT:
                    raw = xq_raw[:, t, :]
                else:
                    raw = gpool.tile([P, c.D], F32, tag="graw", name="graw")[:]
                gather_rows(raw, hid_d, idxkv, t)
                norm_transpose(
                    raw, lambda dt, t=t: xkvT[:, dt, t * P : (t + 1) * P]
                )
            # cos/sin gathers + transposes (f32)
            for srcd, idxt, nt, dst in () if not _ph("B") else (
                (cos_d, idxkv, KT, cosTkv),
                (sin_d, idxkv, KT, sinTkv),
            ):
                for t in range(nt):
                    cg = cpool.tile([P, c.HD], F32, tag="cg")
                    gather_rows(cg[:], srcd, idxt, t)
                    tp = ps_tile()
                    nc.tensor.transpose(tp[:, :P], cg[:], idf[:])
                    nc.scalar.copy(dst[:, t * P : (t + 1) * P], tp[:, :P])

        # ---- phases C+D: projections + rope, interleaved with attention ----
        # Emission order: K proj, V proj, Q(0), Q(1), then per attention head
        # h: Q(h+2) and scores(h+1) are emitted BEFORE AV/ones(h), so the PE
        # queue always has dense independent matmuls to run while the
        # exp->mask chain of the current head completes on scalar/vector.
        es_qkv = ExitStack()
        qkvp = es_qkv.enter_context(tc.tile_pool(name="qkvp", bufs=1, side="right"))
        kT = qkvp.tile([P, c.KVH, c.KSEL], BF16, tag="kT")
        vN = qkvp.tile([P, KT, KVD], BF16, tag="vN")
        qT = qkvp.tile([P, c.H, QROWS], BF16, tag="qT")

        # o-proj weight prefetch: fresh SBUF region + early queue position so
        # chunks stream in during C/D and phase E never waits on weights
        es_wo = ExitStack()
        wop = es_wo.enter_context(tc.tile_pool(name="wop", bufs=4, side="left"))
        wo_tiles = {}
        for ogi in range(len(OGS) if _ph("E") else 0):
            for c4 in range(c.H // 4):
                wt = wop.tile([P, 4, OW], BF16, tag="wot")
                # gpsimd-engine queue: decoupled from the sync queue so these
                # prefetches never head-of-line-block the Q/K weight streams
                nc.gpsimd.dma_start(wt[:], wo_d[ogi, c4])
                wo_tiles[(ogi, c4)] = wt

        es_attn = ExitStack()
        attnp = es_attn.enter_context(tc.tile_pool(name="attnp", bufs=1, side="left"))
        xattnT = attnp.tile([P, c.H, QROWS], BF16, tag="xattnT")

        # Causal structure (keys permuted: own half first):
        #   tile j<4: affects only query cols >= j*128; diagonal 128x128
        #     sub-block is triangular (tri multiply); rest fully visible.
        #   tile j>=4: all-visible (h=1) or all-masked (h=0) -- via biasj.
        # exp computes exp(scl*s + biasj) with biasj in {-1, -30001}: the
        # uniform -1 shift guards fp8 overflow and cancels in the divide.
        # expT is fp8 (softmax weights in [0, e^4.5]); the softmax-sum uses a
        # DoubleRow fp8 matmul against a constant 16.0 stationary (the x16
        # is divided back out in the final normalization).
        def jow(j):
            o = j * P if j < 4 else 0
            return o, QROWS - o

        with tc.tile_pool(name="wstr", bufs=3, side="left") as wstr, tc.tile_pool(name="rpool", bufs=3, side="left") as rpool, tc.tile_pool(name="ropep", bufs=2, side="left") as ropep, tc.tile_pool(name="dpool", bufs=3, side="left") as dpool, tc.tile_pool(name="recp", bufs=3, side="left") as recp:

            def rope(dst, rawt, rot_ps, cosT, sinT, s0, w):
                t1 = ropep.tile([P, 512], F32, tag="ropet1")
                nc.vector.tensor_mul(t1[:, :w], rawt[:, s0 : s0 + w], cosT[:, s0 : s0 + w])
                t2 = ropep.tile([P, 512], F32, tag="ropet2")
                nc.vector.tensor_mul(t2[:, :w], rot_ps[:, :w], sinT[:, s0 : s0 + w])
                nc.vector.tensor_add(dst[:, s0 : s0 + w], t1[:, :w], t2[:, :w])

            def qproj(m):
                wqm = wstr.tile([P, DT, c.HD], BF16, tag="wqkm", name="wqm")
                nc.sync.dma_start(wqm[:], wq_d[m])
                qraw = rpool.tile([P, c.KSEL], BF16, tag="kqraw", name="qraw")
                ps = ps_tile()
                for dt in range(DT):
                    nc.tensor.matmul(
                        ps[:, :QROWS],
                        wqm[:, dt, :],
                        xqT[:, dt, :],
                        start=(dt == 0),
                        stop=(dt == DT - 1),
                    )
                nc.vector.tensor_scalar_add(
                    qraw[:, :QROWS], ps[:, :QROWS], bqc[:, m : m + 1]
                )
                rot = ps_tile()
                nc.tensor.matmul(
                    rot[:, :QROWS], perm[:], qraw[:, :QROWS], start=True, stop=True
                )
                rope(qT[:, m, :], qraw, rot, cosTq, sinTq, 0, QROWS)

            def kproj(m):
                wkm = wstr.tile([P, DT, c.HD], BF16, tag="wqkm", name="wkm")
                nc.sync.dma_start(wkm[:], wk_d[m])
                kraw = rpool.tile([P, c.KSEL], BF16, tag="kqraw")
                for s0, w in _chunks(c.KSEL, 512):
                    ps = ps_tile()
                    for dt in range(DT):
                        nc.tensor.matmul(
                            ps[:, :w],
                            wkm[:, dt, :],
                            xkvT[:, dt, s0 : s0 + w],
                            start=(dt == 0),
                            stop=(dt == DT - 1),
                        )
                    nc.vector.tensor_scalar_add(
                        kraw[:, s0 : s0 + w], ps[:, :w], bkc[:, m : m + 1]
                    )
                for s0, w in _chunks(c.KSEL, 512):
                    rot = ps_tile()
                    nc.tensor.matmul(
                        rot[:, :w], perm[:], kraw[:, s0 : s0 + w], start=True, stop=True
                    )
                    rope(kT[:, m, :], kraw, rot, cosTkv, sinTkv, s0, w)

            exp_tiles = {}

            def scores_block(h):
                g = h // c.GQ
                expT = dpool.tile([P, KT, QROWS], F8, tag="expT")
                exp_tiles[h] = expT
                # zero the fp8 pair-mate gaps (cols outside a tile's causal
                # range that its DoubleRow partner still streams); on vector:
                # the gpsimd queue carries prefetch DMA triggers and would
                # serialize the softmax-sum matmuls behind them
                nc.vector.memset(expT[:, 1, 0:P], 0.0)
                nc.vector.memset(expT[:, 3, 2 * P : 3 * P], 0.0)
                for j in range(KT):
                    o, w = jow(j)
                    ps = ps_tile()
                    nc.tensor.matmul(
                        ps[:, o:QROWS],
                        kT[:, g, j * P : (j + 1) * P],
                        qT[:, h, o:QROWS],
                        start=True,
                        stop=True,
                    )
                    nc.scalar.activation(
                        expT[:, j, o:QROWS], ps[:, o:QROWS], AF.Exp,
                        scale=scl, bias=biasjc[:, j : j + 1],
                    )
                    if j < 4:
                        nc.vector.tensor_mul(
                            expT[:, j, j * P : (j + 1) * P],
                            expT[:, j, j * P : (j + 1) * P],
                            tri[:],
                        )

            def av_ones_drain(h):
                g = h // c.GQ
                expT = exp_tiles.pop(h)
                pso = ps_tile()
                pss = ps_tile()
                for j in range(KT):
                    o, w = jow(j)
                    nc.tensor.matmul(
                        pso[:, o:QROWS],
                        vN[:, j, g * c.HD : (g + 1) * c.HD],
                        expT[:, j, o:QROWS],
                        start=(j == 0),
                        stop=(j == KT - 1),
                        skip_group_check=True,
                    )
                for p2 in range(KT // 2):
                    o, w = jow(2 * p2)
                    nc.tensor.matmul(
                        pss[:, o:QROWS],
                        ones16[:],
                        expT[:, 2 * p2 : 2 * p2 + 2, o:QROWS],
                        start=(p2 == 0),
                        stop=(p2 == KT // 2 - 1),
                        skip_group_check=True,
                        perf_mode=DR,
                    )
                rec = recp.tile([P, QROWS], F32, tag="rec")
                nc.vector.reciprocal_approx_fast(rec[:], pss[:, :QROWS])
                # pss = 16*sum(exp): fold the /16 back via the 16.0 scalar
                nc.vector.scalar_tensor_tensor(
                    xattnT[:, h, :], pso[:, :QROWS], 16.0, rec[:],
                    mybir.AluOpType.mult, mybir.AluOpType.mult,
                )

            # K + V first, then Q, then attention (scores one head ahead)
            with tc.tile_pool(name="wvp", bufs=1, side="left") as wvp:
                wvsb = wvp.tile([P, DT, KVD], BF16, tag="wvsb")
                nc.sync.dma_start(wvsb[:], wv_d[:])
                for m in range(c.KVH if _ph("C") else 0):
                    kproj(m)
                for rt in range(KT if _ph("C") else 0):
                    psv = ps_tile()
                    for dt in range(DT):
                        nc.tensor.matmul(
                            psv[:, :KVD],
                            xkvT[:, dt, rt * P : (rt + 1) * P],
                            wvsb[:, dt, :],
                            start=(dt == 0),
                            stop=(dt == DT - 1),
                        )
                    nc.vector.tensor_add(vN[:, rt, :], psv[:, :KVD], bvbc[:])

            for m in range(c.H if _ph("C") else 0):
                qproj(m)

            if _ph("D"):
                scores_block(0)
                for h in range(c.H):
                    if h + 1 < c.H:
                        scores_block(h + 1)
                    av_ones_drain(h)

        es_qkv.close()  # free kT/vN/qT (right side)

        # attn+mlp residual, lives E -> G
        es_res2 = ExitStack()
        res2p = es_res2.enter_context(tc.tile_pool(name="res2p", bufs=1, side="right"))
        res2 = res2p.tile([P, QT, c.D], F32, tag="res2")

        # ---- phase E: o-proj + residual + rmsnorm2 ----
        es_xm = ExitStack()
        xmp = es_xm.enter_context(tc.tile_pool(name="xmp", bufs=1, side="right"))
        xmT = xmp.tile([P, DT, QROWS], BF16, tag="xmT")

        with tc.tile_pool(name="gpool2", bufs=3, side="left") as gpool2, tc.tile_pool(name="spool2", bufs=4, side="left") as spool2:
            def _norm2_transpose(qt):
                mv = spool2.tile([P, 2], F32, tag="mv2", name="mv")
                nc.vector.bn_aggr(mv[:], stats2[:, qt])
                msq = spool2.tile([P, 1], F32, tag="msq2", name="msq")
                nc.vector.tensor_mul(msq[:], mv[:, 0:1], mv[:, 0:1])
                nc.vector.tensor_add(msq[:], msq[:], mv[:, 1:2])
                srt = spool2.tile([P, 1], F32, tag="srt2", name="srt")
                nc.scalar.activation(srt[:], msq[:], AF.Sqrt, bias=epsc[:])
                rstd = spool2.tile([P, 1], F32, tag="rstd2", name="rstd")
                nc.vector.reciprocal(rstd[:], srt[:])
                xn = gpool2.tile([P, c.D], BF16, tag="xn2", name="xn")
                nc.vector.tensor_scalar_mul(xn[:], res2[:, qt, :], rstd[:])
                for dt in range(DT):
                    tp = psb_tile()
                    nc.tensor.transpose(
                        tp[:], xn[:, dt * P : (dt + 1) * P], idb[:]
                    )
                    nc.scalar.copy(xmT[:, dt, qt * P : (qt + 1) * P], tp[:])

            stats2 = spool2.tile([P, QT, nsub, 6], F32, tag="stats2all")
            for ogi, (os_, ow) in enumerate(OGS if _ph("E") else []):
                pss4 = [ps_tile() for _ in range(QT)]
                for c4 in range(c.H // 4):
                    wot = wo_tiles[(ogi, c4)]
                    for i in range(4):
                        ht = c4 * 4 + i
                        for qt in range(QT):
                            nc.tensor.matmul(
                                pss4[qt][:, :ow],
                                xattnT[:, ht, qt * P : (qt + 1) * P],
                                wot[:, i, :ow],
                                start=(ht == 0),
                                stop=(ht == c.H - 1),
                            )
                for qt in range(QT):
                    nc.vector.tensor_add(
                        res2[:, qt, os_ : os_ + ow],
                        pss4[qt][:, :ow],
                        xq_raw[:, qt, os_ : os_ + ow],
                    )
                    # incremental norm2 stats: OGS chunks == bn subgroups
                    nc.vector.bn_stats(
                        stats2[:, qt, ogi, :], res2[:, qt, os_ : os_ + ow]
                    )
            for qt in range(QT if _ph("E") else 0):
                _norm2_transpose(qt)
            for qt in range(QT if _ph("E") else 0):
                # fold gating: res2 := (res2 - xq_raw)*g + xq_raw, so phase G
                # only needs one fused op per output tile after the last MM
                nc.vector.tensor_sub(
                    res2[:, qt, :], res2[:, qt, :], xq_raw[:, qt, :]
                )
                nc.vector.tensor_scalar_mul(
                    res2[:, qt, :], res2[:, qt, :], gsc[:, qt : qt + 1]
                )
                nc.vector.tensor_add(
                    res2[:, qt, :], res2[:, qt, :], xq_raw[:, qt, :]
                )

        es_attn.close()  # free xattnT
        es_wo.close()    # free wo tiles
        es_bt.close()    # free xkvT/xqT/cos/sin (held through D for Q-proj)
        es_xq.close()    # free xq_raw (gating already folded into res2)

        # ---- phase F: mlp gate/up ----
        es_act = ExitStack()
        actp = es_act.enter_context(tc.tile_pool(name="actp", bufs=1, side="left"))
        actT = actp.tile([P, c.FFT, QROWS], BF16, tag="actT")

        es_wd = ExitStack()  # down-proj weight stream: spans F (prefetch) + G
        wstr4 = es_wd.enter_context(tc.tile_pool(name="wstr4", bufs=6, side="left"))
        wd_pre = []
        for pi in range(2 if _ph("G") else 0):
            wdt = wstr4.tile([P, 4, OW], BF16, tag="wdt")
            nc.gpsimd.dma_start(wdt[:], wd_d[0, pi])
            wd_pre.append(wdt)

        with tc.tile_pool(name="wstr3", bufs=6, side="left") as wstr3, tc.tile_pool(name="fpool", bufs=3, side="left") as fpool:
            for g in range(c.FFG if _ph("F") else 0):
                psg = [ps_tile() for _ in range(4)]
                for d4 in range(DT // 4):
                    wgt = wstr3.tile([P, 4, 512], BF16, tag="wgut")
                    nc.sync.dma_start(wgt[:], wg_d[g, d4])
                    for i in range(4):
                        dt = d4 * 4 + i
                        for s in range(4):
                            nc.tensor.matmul(
                                psg[s][:, :QROWS],
                                wgt[:, i, s * P : (s + 1) * P],
                                xmT[:, dt, :],
                                start=(dt == 0),
                                stop=(dt == DT - 1),
                            )
                silu = fpool.tile([P, 4, QROWS], F32, tag="silu")
                for s in range(4):
                    # silu(x) = x * sigmoid(x) (Silu isn't in CoreSim)
                    nc.scalar.activation(silu[:, s, :], psg[s][:, :QROWS], AF.Sigmoid)
                    nc.vector.tensor_mul(silu[:, s, :], silu[:, s, :], psg[s][:, :QROWS])
                psu = [ps_tile() for _ in range(4)]
                for d4 in range(DT // 4):
                    wut = wstr3.tile([P, 4, 512], BF16, tag="wgut")
                    nc.sync.dma_start(wut[:], wu_d[g, d4])
                    for i in range(4):
                        dt = d4 * 4 + i
                        for s in range(4):
                            nc.tensor.matmul(
                                psu[s][:, :QROWS],
                                wut[:, i, s * P : (s + 1) * P],
                                xmT[:, dt, :],
                                start=(dt == 0),
                                stop=(dt == DT - 1),
                            )
                for s in range(4):
                    nc.vector.tensor_mul(
                        actT[:, g * 4 + s, :], silu[:, s, :], psu[s][:, :QROWS]
                    )

        es_xm.close()  # free xmT

        # ---- phase G: down-proj + residual + gating + output ----
        with tc.tile_pool(name="opool", bufs=3, side="left") as opool:
            for ogi, (os_, ow) in enumerate(OGS if _ph("G") else []):
                psd = [ps_tile() for _ in range(QT)]
                for f4 in range(c.FFT // 4):
                    if ogi == 0 and f4 < 2:
                        wdt = wd_pre[f4]
                    else:
                        wdt = wstr4.tile([P, 4, OW], BF16, tag="wdt")
                        nc.sync.dma_start(wdt[:], wd_d[ogi, f4])
                    for i in range(4):
                        ffp = f4 * 4 + i
                        for qt in range(QT):
                            nc.tensor.matmul(
                                psd[qt][:, :ow],
                                actT[:, ffp, qt * P : (qt + 1) * P],
                                wdt[:, i, :ow],
                                start=(ffp == 0),
                                stop=(ffp == c.FFT - 1),
                            )
                for qt in range(QT):
                    t1 = opool.tile([P, 512], F32, tag="updt")
                    nc.vector.scalar_tensor_tensor(
                        t1[:, :ow],
                        psd[qt][:, :ow],
                        gsc[:, qt : qt + 1],
                        res2[:, qt, os_ : os_ + ow],
                        mybir.AluOpType.mult,
                        mybir.AluOpType.add,
                    )
                    nc.sync.dma_start(
                        oupd_d[qt * P : (qt + 1) * P, os_ : os_ + ow], t1[:, :ow]
                    )

        es_wd.close()
        es_act.close()
        es_res2.close()
    return nc


# ---------------- host side ----------------


def _bf(x):
    return np.ascontiguousarray(x.astype(BF16NP))


def _f32(x):
    return np.ascontiguousarray(x, dtype=np.float32)


def prep_shared(c: Cfg, Wq, bq, Wk, bk, Wv, bv, Wo, w_gate, w_up, w_down, ln1_w, ln2_w):
    """Host-side weight folding + tiling (exact fp32 math, then bf16 cast)."""
    DT, FFT, FFG, KVD = c.DT, c.FFT, c.FFG, c.KVH * c.HD
    OGS = _chunks(c.D, 512)
    OG, OW = len(OGS), OGS[0][1]
    Wqf = _f32(Wq) * _f32(ln1_w)[:, None]
    Wkf = _f32(Wk) * _f32(ln1_w)[:, None]
    Wvf = _f32(Wv) * _f32(ln1_w)[:, None]
    Wgf = _f32(w_gate) * _f32(ln2_w)[:, None]
    Wuf = _f32(w_up) * _f32(ln2_w)[:, None]

    perm = np.zeros((P, P), np.float32)
    half = c.HD // 2
    perm[np.arange(half) + half, np.arange(half)] = -1.0
    perm[np.arange(half), np.arange(half) + half] = 1.0

    # tri[k, q] = 1 if k <= q (keep) else 0, for the diagonal 128x128 block
    tri = np.triu(np.ones((P, P), np.float32))

    return dict(
        wq=_bf(Wqf.reshape(DT, P, c.H, c.HD).transpose(2, 1, 0, 3)),
        wk=_bf(Wkf.reshape(DT, P, c.KVH, c.HD).transpose(2, 1, 0, 3)),
        wv=_bf(Wvf.reshape(DT, P, KVD).transpose(1, 0, 2)),
        # wo[ogi, c4, p, i, col] = Wo[(c4*4+i)*128+p, ogi*512+col]
        wo=_bf(_f32(Wo).reshape(c.H // 4, 4, P, OG, OW).transpose(3, 0, 2, 1, 4)),
        # wg[g, d4, p, i, col] = Wgf[(d4*4+i)*128+p, g*512+col]
        wg=_bf(Wgf.reshape(DT // 4, 4, P, FFG, 512).transpose(3, 0, 2, 1, 4)),
        wu=_bf(Wuf.reshape(DT // 4, 4, P, FFG, 512).transpose(3, 0, 2, 1, 4)),
        # wd[ogi, f4, p, i, col] = w_down[(f4*4+i)*128+p, ogi*512+col]
        wd=_bf(_f32(w_down).reshape(FFT // 4, 4, P, OG, OW).transpose(3, 0, 2, 1, 4)),
        bq=_f32(bq).reshape(c.H, P, 1),
        bk=_f32(bk).reshape(c.KVH, P, 1),
        bv=_f32(bv).reshape(1, KVD),
        id_f=np.eye(P, dtype=np.float32),
        id_b=np.eye(P, dtype=np.float32).astype(BF16NP),
        perm=perm.astype(BF16NP),
        ones16=np.full((P, 2, P), 16.0, np.float32).astype(F8NP),
        tri=tri.astype(F8NP),
    )


def prep_core(c: Cfg, shared, hid_b, idx_b, g_b, cos_b, sin_b, h):
    """Per-core inputs for core handling query-half h of one batch."""
    QROWS, QT, KT = c.QROWS, c.QT, c.KT
    idx32 = idx_b.astype(np.int32)
    # permute keys so this core's own query half comes first; block-causal
    # masking is then uniform: tile j<4 affects only cols >= j*128 with a
    # triangular diagonal block; tiles j>=4 are all-or-nothing via biasj
    kperm = np.concatenate(
        [np.arange(h * QROWS, (h + 1) * QROWS),
         np.arange(0, h * QROWS), np.arange((h + 1) * QROWS, c.KSEL)]
    )
    idx32 = idx32[kperm]
    # exp bias per key tile: -1 everywhere (overflow guard, cancels in the
    # normalization); other-half tiles fully masked for h=0 cores
    biasj = np.full((P, KT), -1.0, np.float32)
    if h == 0:
        biasj[:, 4:] = -30001.0
    m = dict(
        hid=_f32(hid_b),
        idx_kv=np.ascontiguousarray(idx32.reshape(KT, P).T),
        gsc=np.ascontiguousarray(
            _f32(g_b[h * QROWS : (h + 1) * QROWS]).reshape(QT, P).T
        ),
        cosb=_f32(cos_b),
        sinb=_f32(sin_b),
        biasj=biasj,
    )
    m.update(shared)
    return m


_NC_CACHE = {}


def _get_nc(c: Cfg):
    key = c
    if key not in _NC_CACHE:
        nc = bacc.Bacc()
        emit(nc, c)
        nc.compile()
        _NC_CACHE[key] = nc
    return _NC_CACHE[key]


_RUN_CACHE = {}


def _run_spmd_cached(c: Cfg, nc, in_maps):
    """run_bass_via_pjrt equivalent with a cached jitted executable.

    run_bass_kernel_spmd rebuilds its jit closure per call, so every kernel()
    invocation would re-trace + recompile (~40s).  Build the shard_map jit
    once per config and reuse it; repeat calls only pay host->device
    transfer + execution.
    """
    import jax
    import numpy as np
    from jax.sharding import Mesh, PartitionSpec
    from jax.experimental.shard_map import shard_map
    from concourse import bass2jax
    from concourse.bass2jax import _bass_exec_p, install_neuronx_cc_hook

    n_cores = len(in_maps)
    key = (c, n_cores)
    if key not in _RUN_CACHE:
        install_neuronx_cc_hook()
        partition_name = (
            nc.partition_id_tensor.name if nc.partition_id_tensor else None
        )
        in_names, out_names, out_avals = [], [], []
        for alloc in nc.m.functions[0].allocations:
            if not isinstance(alloc, mybir.MemoryLocationSet):
                continue
            name = alloc.memorylocations[0].name
            if alloc.kind == "ExternalInput":
                if name != partition_name:
                    in_names.append(name)
            elif alloc.kind == "ExternalOutput":
                out_names.append(name)
                out_avals.append(
                    jax.core.ShapedArray(
                        tuple(alloc.tensor_shape), mybir.dt.np(alloc.dtype)
                    )
                )
        n_params = len(in_names)
        all_in = list(in_names) + list(out_names)
        if partition_name is not None:
            all_in.append(partition_name)

        def _body(*flat):
            operands = list(flat)
            if partition_name is not None:
                operands.append(bass2jax.partition_id_tensor())
            return tuple(
                _bass_exec_p.bind(
                    *operands,
                    out_avals=tuple(out_avals),
                    in_names=tuple(all_in),
                    out_names=tuple(out_names),
                    lowering_input_output_aliases=(),
                    sim_require_finite=True,
                    sim_require_nnan=True,
                    nc=nc,
                )
            )

        devices = jax.devices()[:n_cores]
        mesh = Mesh(np.asarray(devices), ("core",))
        n_outs = len(out_avals)
        sharded = jax.jit(
            shard_map(
                _body,
                mesh=mesh,
                in_specs=(PartitionSpec("core"),) * (n_params + n_outs),
                out_specs=(PartitionSpec("core"),) * n_outs,
                check_rep=False,
            ),
            keep_unused=True,
        )
        zeros = [
            np.zeros((n_cores * a.shape[0], *a.shape[1:]), a.dtype)
            for a in out_avals
        ]
        _RUN_CACHE[key] = (sharded, in_names, out_names, out_avals, zeros)

    sharded, in_names, out_names, out_avals, zeros = _RUN_CACHE[key]
    concat_in = [
        np.concatenate([np.asarray(in_maps[ci][nm]) for ci in range(n_cores)], axis=0)
        for nm in in_names
    ]
    out_arrs = sharded(*concat_in, *zeros)
    return [
        {
            name: np.asarray(out_arrs[i]).reshape(n_cores, *out_avals[i].shape)[ci]
            for i, name in enumerate(out_names)
        }
        for ci in range(n_cores)
    ]


def assemble_output(inputs, res):
    """Build the full [B, T, D] output from per-core result maps."""
    c = FULL
    hidden_states = np.asarray(inputs["hidden_states"])
    topk = np.asarray(inputs["topk_indices"])
    B = hidden_states.shape[0]
    final = np.ascontiguousarray(hidden_states, dtype=np.float32).copy()
    for ci in range(2 * B):
        b, h = ci // 2, ci % 2
        sel = topk[b, h * c.QROWS : (h + 1) * c.QROWS].astype(np.int64)
        final[b, sel] = res[ci]["out_upd"]
    return final


def kernel(
    hidden_states,
    topk_indices,
    gating_scores,
    cos,
    sin,
    Wq,
    bq,
    Wk,
    bk,
    Wv,
    bv,
    Wo,
    w_gate,
    w_up,
    w_down,
    ln1_w,
    ln2_w,
):
    c = FULL
    B = hidden_states.shape[0]
    hidden_states = np.asarray(hidden_states)
    topk_indices = np.asarray(topk_indices)
    shared = prep_shared(
        c, Wq, bq, Wk, bk, Wv, bv, Wo, w_gate, w_up, w_down, ln1_w, ln2_w
    )
    in_maps = []
    for b in range(B):
        for h in range(2):
            in_maps.append(
                prep_core(
                    c,
                    shared,
                    hidden_states[b],
                    topk_indices[b],
                    np.asarray(gating_scores)[b],
                    np.asarray(cos)[b],
                    np.asarray(sin)[b],
                    h,
                )
            )
    nc = _get_nc(c)
    res = _run_spmd_cached(c, nc, in_maps)

    return assemble_output(
        dict(hidden_states=hidden_states, topk_indices=topk_indices), res
    )


# revision 30
# speedup vs baseline: 1.1884x; 1.0031x over previous
"""Trainium2 Bass kernel for the DynamicBlock (ragged top-k decoder layer).

Sharding: 8 cores = (batch b in 0..3) x (query-half h in 0..1).
Core (b, h) processes queries k in [h*512, (h+1)*512) of the K=1024 selected
rows of batch b (causal: needs K/V for all 1024 selected rows, computed
locally -- no collectives).  Untouched hidden rows are assembled host-side.
Matmuls run in bf16 with fp32 accumulation; norms/softmax/residual/gating
in fp32.

Attention exploits causality uniformly across cores: keys are permuted so
this core's own query half is tiles j=0..3 (block-triangular: tile j only
affects query columns >= j*128) and the other half is tiles j=4..7, which
are either fully visible (h=1) or fully masked (h=0) -- expressed as a
per-core additive bias on the exp, so one program serves both core types.
"""

import math
from contextlib import ExitStack
from dataclasses import dataclass

import ml_dtypes
import numpy as np

import concourse.bass as bass
import concourse.mybir as mybir
import concourse.tile as tile
from concourse import bacc
from concourse.bass import IndirectOffsetOnAxis

P = 128
F32 = mybir.dt.float32
BF16 = mybir.dt.bfloat16
F8 = mybir.dt.float8e4
I32 = mybir.dt.int32
AF = mybir.ActivationFunctionType
BF16NP = ml_dtypes.bfloat16
F8NP = ml_dtypes.float8_e4m3
DR = mybir.MatmulPerfMode.DoubleRow


@dataclass(frozen=True)
class Cfg:
    T: int = 4096      # full sequence length
    D: int = 2048      # model dim
    KSEL: int = 1024   # selected rows per sequence
    H: int = 16        # query heads
    KVH: int = 4       # kv heads
    HD: int = 128      # head dim (must equal P)
    FF: int = 8192     # mlp intermediate
    EPS: float = 1e-6

    @property
    def DT(self):
        return self.D // P

    @property
    def QROWS(self):
        return self.KSEL // 2

    @property
    def QT(self):
        return self.QROWS // P

    @property
    def KT(self):
        return self.KSEL // P

    @property
    def FFT(self):
        return self.FF // P

    @property
    def FFG(self):
        return self.FFT // 4

    @property
    def T2(self):
        return self.T // 2

    @property
    def GQ(self):
        return self.H // self.KVH


FULL = Cfg()


def _chunks(total, size):
    out = []
    s = 0
    while s < total:
        out.append((s, min(size, total - s)))
        s += size
    return out


def emit(nc: bass.Bass, c: Cfg, upto: str = "G"):
    _PH = "ABCDEFG"

    def _ph(p):
        return _PH.index(p) <= _PH.index(upto)

    DT, QT, KT, QROWS, KVD = c.DT, c.QT, c.KT, c.QROWS, c.KVH * c.HD
    OGS = _chunks(c.D, 512)  # output-column groups for o-proj / down-proj
    OW = OGS[0][1]

    # ---- DRAM I/O ----
    hid_d = nc.dram_tensor("hid", [c.T, c.D], F32, kind="ExternalInput")
    idxkv_d = nc.dram_tensor("idx_kv", [P, KT], I32, kind="ExternalInput")
    gsc_d = nc.dram_tensor("gsc", [P, QT], F32, kind="ExternalInput")
    cos_d = nc.dram_tensor("cosb", [c.T, c.HD], F32, kind="ExternalInput")
    sin_d = nc.dram_tensor("sinb", [c.T, c.HD], F32, kind="ExternalInput")
    biasj_d = nc.dram_tensor("biasj", [P, KT], F32, kind="ExternalInput")
    tri_d = nc.dram_tensor("tri", [P, P], F8, kind="ExternalInput")
    wq_d = nc.dram_tensor("wq", [c.H, P, DT, c.HD], BF16, kind="ExternalInput")
    wk_d = nc.dram_tensor("wk", [c.KVH, P, DT, c.HD], BF16, kind="ExternalInput")
    wv_d = nc.dram_tensor("wv", [P, DT, KVD], BF16, kind="ExternalInput")
    wo_d = nc.dram_tensor("wo", [len(OGS), c.H // 4, P, 4, OW], BF16, kind="ExternalInput")
    wg_d = nc.dram_tensor("wg", [c.FFG, DT // 4, P, 4, 512], BF16, kind="ExternalInput")
    wu_d = nc.dram_tensor("wu", [c.FFG, DT // 4, P, 4, 512], BF16, kind="ExternalInput")
    wd_d = nc.dram_tensor("wd", [len(OGS), c.FFT // 4, P, 4, OW], BF16, kind="ExternalInput")
    bq_d = nc.dram_tensor("bq", [c.H, P, 1], F32, kind="ExternalInput")
    bk_d = nc.dram_tensor("bk", [c.KVH, P, 1], F32, kind="ExternalInput")
    bv_d = nc.dram_tensor("bv", [1, KVD], F32, kind="ExternalInput")
    idf_d = nc.dram_tensor("id_f", [P, P], F32, kind="ExternalInput")
    idb_d = nc.dram_tensor("id_b", [P, P], BF16, kind="ExternalInput")
    perm_d = nc.dram_tensor("perm", [P, P], BF16, kind="ExternalInput")
    ones_d = nc.dram_tensor("ones16", [P, 2, P], F8, kind="ExternalInput")

    oupd_d = nc.dram_tensor("out_upd", [QROWS, c.D], F32, kind="ExternalOutput")

    scl = 1.0 / math.sqrt(c.HD)

    with ExitStack() as top:
        tc = top.enter_context(tile.TileContext(nc))
        constp = top.enter_context(tc.tile_pool(name="constp", bufs=1, side="left"))
        residp = top.enter_context(tc.tile_pool(name="residp", bufs=1, side="left"))
        psp = top.enter_context(tc.tile_pool(name="psp", bufs=6, space="PSUM"))
        psbp = top.enter_context(tc.tile_pool(name="psbp", bufs=2, space="PSUM"))

        def ps_tile():
            return psp.tile([P, 512], F32, tag="ps", name="ps")

        def psb_tile():
            return psbp.tile([P, P], BF16, tag="psb", name="psb")

        # ---- constants (indices first: they gate the gathers) ----
        idxkv = constp.tile([P, KT], I32, tag="idxkv")
        nc.sync.dma_start(idxkv[:], idxkv_d[:])
        idf = constp.tile([P, P], F32, tag="idf")
        nc.sync.dma_start(idf[:], idf_d[:])
        idb = constp.tile([P, P], BF16, tag="idb")
        nc.sync.dma_start(idb[:], idb_d[:])
        perm = constp.tile([P, P], BF16, tag="perm")
        nc.sync.dma_start(perm[:], perm_d[:])
        ones16 = constp.tile([P, 2, P], F8, tag="ones16")
        nc.sync.dma_start(ones16[:], ones_d[:])
        tri = constp.tile([P, P], F8, tag="tri")
        nc.sync.dma_start(tri[:], tri_d[:])
        gsc = constp.tile([P, QT], F32, tag="gsc")
        nc.sync.dma_start(gsc[:], gsc_d[:])
        biasjc = constp.tile([P, KT], F32, tag="biasjc")
        nc.sync.dma_start(biasjc[:], biasj_d[:])
        bqc = constp.tile([P, c.H], F32, tag="bqc")
        for m in range(c.H):
            nc.sync.dma_start(bqc[:, m : m + 1], bq_d[m])
        bkc = constp.tile([P, c.KVH], F32, tag="bkc")
        for m in range(c.KVH):
            nc.sync.dma_start(bkc[:, m : m + 1], bk_d[m])
        epsc = constp.tile([P, 1], F32, tag="epsc")
        nc.vector.memset(epsc[:], c.EPS)
        bvbc = constp.tile([P, KVD], F32, tag="bvbc")
        bv_ap = bv_d[:]
        nc.sync.dma_start(
            bvbc[:], bass.AP(tensor=bv_ap.tensor, offset=0, ap=[[0, P], [1, KVD]])
        )

        # first-residual (lives B -> E; freed before the MLP phases)
        es_xq = ExitStack()
        xqp = es_xq.enter_context(tc.tile_pool(name="xqp", bufs=1, side="left"))
        xq_raw = xqp.tile([P, QT, c.D], F32, tag="xq_raw")

        sgw = math.gcd(512, c.D)
        nsub = c.D // sgw

        es_bt = ExitStack()  # xkvT/xqT/cos/sin: freed after projections
        xtp = es_bt.enter_context(tc.tile_pool(name="xtp", bufs=1, side="left"))
        xkvT = xtp.tile([P, DT, c.KSEL], BF16, tag="xkvT")
        cosTkv = xtp.tile([P, c.KSEL], F32, tag="cosTkv")
        sinTkv = xtp.tile([P, c.KSEL], F32, tag="sinTkv")
        # host permutes the key order so this core's own query half is rows
        # [0, QROWS) -- q-side tensors are static slices of the kv tensors
        xqT = xkvT[:, :, :QROWS]
        cosTq = cosTkv[:, :QROWS]
        sinTq = sinTkv[:, :QROWS]

        # ---- phase B: gather + rmsnorm1 + transpose ----
        def gather_rows(dst, src_dram, idx_tile, col):
            """Indirect row gather with a gpsimd shield op.

            The shield write/read absorbs the WAR (slot reuse) and RAW (index
            load) waits into a compute op on the triggering engine -- the
            dynamic-queue DMA itself only supports a single sync wait.
            """
            nc.gpsimd.tensor_copy(dst[0:1, 0:1], idx_tile[0:1, col : col + 1])
            nc.gpsimd.indirect_dma_start(
                out=dst,
                out_offset=None,
                in_=src_dram[:],
                in_offset=IndirectOffsetOnAxis(ap=idx_tile[:, col : col + 1], axis=0),
            )

        def norm_transpose(raw, xn_out_fn):
            """raw: [P, D] f32 tile; writes bf16 normalized transposed tiles."""
            stats = spool.tile([P, nsub, 6], F32, tag="stats")
            for s in range(nsub):
                nc.vector.bn_stats(stats[:, s, :], raw[:, s * sgw : (s + 1) * sgw])
            mv = spool.tile([P, 2], F32, tag="mv")
            nc.vector.bn_aggr(mv[:], stats[:])
            msq = spool.tile([P, 1], F32, tag="msq")
            nc.vector.tensor_mul(msq[:], mv[:, 0:1], mv[:, 0:1])
            nc.vector.tensor_add(msq[:], msq[:], mv[:, 1:2])
            srt = spool.tile([P, 1], F32, tag="srt")
            nc.scalar.activation(srt[:], msq[:], AF.Sqrt, bias=epsc[:])
            rstd = spool.tile([P, 1], F32, tag="rstd")
            nc.vector.reciprocal(rstd[:], srt[:])
            xn = gpool.tile([P, c.D], BF16, tag="xn")
            nc.vector.tensor_scalar_mul(xn[:], raw[:], rstd[:])
            for dt in range(DT):
                tp = psb_tile()
                nc.tensor.transpose(tp[:], xn[:, dt * P : (dt + 1) * P], idb[:])
                nc.scalar.copy(xn_out_fn(dt), tp[:])

        with tc.tile_pool(name="gpool", bufs=4, side="left") as gpool, tc.tile_pool(name="spool", bufs=4, side="left") as spool, tc.tile_pool(name="cpool", bufs=3, side="left") as cpool:
            for t in range(KT if _ph("B") else 0):
                if t < QT:
                    raw = xq_raw[:, t, :]
                else:
                    raw = gpool.tile([P, c.D], F32, tag="graw", name="graw")[:]
                gather_rows(raw, hid_d, idxkv, t)
                norm_transpose(
                    raw, lambda dt, t=t: xkvT[:, dt, t * P : (t + 1) * P]
                )
            # cos/sin gathers + transposes (f32)
            for srcd, idxt, nt, dst in () if not _ph("B") else (
                (cos_d, idxkv, KT, cosTkv),
                (sin_d, idxkv, KT, sinTkv),
            ):
                for t in range(nt):
                    cg = cpool.tile([P, c.HD], F32, tag="cg")
                    gather_rows(cg[:], srcd, idxt, t)
                    tp = ps_tile()
                    nc.tensor.transpose(tp[:, :P], cg[:], idf[:])
                    nc.scalar.copy(dst[:, t * P : (t + 1) * P], tp[:, :P])

        # ---- phases C+D: projections + rope, interleaved with attention ----
        # Emission order: K proj, V proj, Q(0), Q(1), then per attention head
        # h: Q(h+2) and scores(h+1) are emitted BEFORE AV/ones(h), so the PE
        # queue always has dense independent matmuls to run while the
        # exp->mask chain of the current head completes on scalar/vector.
        es_qkv = ExitStack()
        qkvp = es_qkv.enter_context(tc.tile_pool(name="qkvp", bufs=1, side="right"))
        kT = qkvp.tile([P, c.KVH, c.KSEL], BF16, tag="kT")
        vN = qkvp.tile([P, KT, KVD], BF16, tag="vN")
        qT = qkvp.tile([P, c.H, QROWS], BF16, tag="qT")

        # o-proj weight prefetch: fresh SBUF region + early queue position so
        # chunks stream in during C/D and phase E never waits on weights
        es_wo = ExitStack()
        wop = es_wo.enter_context(tc.tile_pool(name="wop", bufs=4, side="left"))
        wo_tiles = {}
        for ogi in range(len(OGS) if _ph("E") else 0):
            for c4 in range(c.H // 4):
                wt = wop.tile([P, 4, OW], BF16, tag="wot")
                # gpsimd-engine queue: decoupled from the sync queue so these
                # prefetches never head-of-line-block the Q/K weight streams
                nc.gpsimd.dma_start(wt[:], wo_d[ogi, c4])
                wo_tiles[(ogi, c4)] = wt

        es_attn = ExitStack()
        attnp = es_attn.enter_context(tc.tile_pool(name="attnp", bufs=1, side="left"))
        xattnT = attnp.tile([P, c.H, QROWS], BF16, tag="xattnT")

        # Causal structure (keys permuted: own half first):
        #   tile j<4: affects only query cols >= j*128; diagonal 128x128
        #     sub-block is triangular (tri multiply); rest fully visible.
        #   tile j>=4: all-visible (h=1) or all-masked (h=0) -- via biasj.
        # exp computes exp(scl*s + biasj) with biasj in {-1, -30001}: the
        # uniform -1 shift guards fp8 overflow and cancels in the divide.
        # expT is fp8 (softmax weights in [0, e^4.5]); the softmax-sum uses a
        # DoubleRow fp8 matmul against a constant 16.0 stationary (the x16
        # is divided back out in the final normalization).
        def jow(j):
            o = j * P if j < 4 else 0
            return o, QROWS - o

        with tc.tile_pool(name="wstr", bufs=3, side="left") as wstr, tc.tile_pool(name="rpool", bufs=3, side="left") as rpool, tc.tile_pool(name="ropep", bufs=2, side="left") as ropep, tc.tile_pool(name="dpool", bufs=3, side="left") as dpool, tc.tile_pool(name="recp", bufs=3, side="left") as recp:

            def rope(dst, rawt, rot_ps, cosT, sinT, s0, w):
                t1 = ropep.tile([P, 512], F32, tag="ropet1")
                nc.vector.tensor_mul(t1[:, :w], rawt[:, s0 : s0 + w], cosT[:, s0 : s0 + w])
                t2 = ropep.tile([P, 512], F32, tag="ropet2")
                nc.vector.tensor_mul(t2[:, :w], rot_ps[:, :w], sinT[:, s0 : s0 + w])
                nc.vector.tensor_add(dst[:, s0 : s0 + w], t1[:, :w], t2[:, :w])

            def qproj(m):
                wqm = wstr.tile([P, DT, c.HD], BF16, tag="wqkm", name="wqm")
                nc.sync.dma_start(wqm[:], wq_d[m])
                qraw = rpool.tile([P, c.KSEL], BF16, tag="kqraw", name="qraw")
                ps = ps_tile()
                for dt in range(DT):
                    nc.tensor.matmul(
                        ps[:, :QROWS],
                        wqm[:, dt, :],
                        xqT[:, dt, :],
                        start=(dt == 0),
                        stop=(dt == DT - 1),
                    )
                nc.vector.tensor_scalar_add(
                    qraw[:, :QROWS], ps[:, :QROWS], bqc[:, m : m + 1]
                )
                rot = ps_tile()
                nc.tensor.matmul(
                    rot[:, :QROWS], perm[:], qraw[:, :QROWS], start=True, stop=True
                )
                rope(qT[:, m, :], qraw, rot, cosTq, sinTq, 0, QROWS)

            def kproj(m):
                wkm = wstr.tile([P, DT, c.HD], BF16, tag="wqkm", name="wkm")
                nc.sync.dma_start(wkm[:], wk_d[m])
                kraw = rpool.tile([P, c.KSEL], BF16, tag="kqraw")
                for s0, w in _chunks(c.KSEL, 512):
                    ps = ps_tile()
                    for dt in range(DT):
                        nc.tensor.matmul(
                            ps[:, :w],
                            wkm[:, dt, :],
                            xkvT[:, dt, s0 : s0 + w],
                            start=(dt == 0),
                            stop=(dt == DT - 1),
                        )
                    nc.vector.tensor_scalar_add(
                        kraw[:, s0 : s0 + w], ps[:, :w], bkc[:, m : m + 1]
                    )
                for s0, w in _chunks(c.KSEL, 512):
                    rot = ps_tile()
                    nc.tensor.matmul(
                        rot[:, :w], perm[:], kraw[:, s0 : s0 + w], start=True, stop=True
                    )
                    rope(kT[:, m, :], kraw, rot, cosTkv, sinTkv, s0, w)

            exp_tiles = {}

            def scores_block(h):
                g = h // c.GQ
                expT = dpool.tile([P, KT, QROWS], F8, tag="expT")
                exp_tiles[h] = expT
                # zero the fp8 pair-mate gaps (cols outside a tile's causal
                # range that its DoubleRow partner still streams); on vector:
                # the gpsimd queue carries prefetch DMA triggers and would
                # serialize the softmax-sum matmuls behind them
                nc.vector.memset(expT[:, 1, 0:P], 0.0)
                nc.vector.memset(expT[:, 3, 2 * P : 3 * P], 0.0)
                for j in range(KT):
                    o, w = jow(j)
                    ps = ps_tile()
                    nc.tensor.matmul(
                        ps[:, o:QROWS],
                        kT[:, g, j * P : (j + 1) * P],
                        qT[:, h, o:QROWS],
                        start=True,
                        stop=True,
                    )
                    nc.scalar.activation(
                        expT[:, j, o:QROWS], ps[:, o:QROWS], AF.Exp,
                        scale=scl, bias=biasjc[:, j : j + 1],
                    )
                    if j < 4:
                        nc.vector.tensor_mul(
                            expT[:, j, j * P : (j + 1) * P],
                            expT[:, j, j * P : (j + 1) * P],
                            tri[:],
                        )

            def av_ones_drain(h):
                g = h // c.GQ
                expT = exp_tiles.pop(h)
                pso = ps_tile()
                pss = ps_tile()
                for j in range(KT):
                    o, w = jow(j)
                    nc.tensor.matmul(
                        pso[:, o:QROWS],
                        vN[:, j, g * c.HD : (g + 1) * c.HD],
                        expT[:, j, o:QROWS],
                        start=(j == 0),
                        stop=(j == KT - 1),
                        skip_group_check=True,
                    )
                for p2 in range(KT // 2):
                    o, w = jow(2 * p2)
                    nc.tensor.matmul(
                        pss[:, o:QROWS],
                        ones16[:],
                        expT[:, 2 * p2 : 2 * p2 + 2, o:QROWS],
                        start=(p2 == 0),
                        stop=(p2 == KT // 2 - 1),
                        skip_group_check=True,
                        perf_mode=DR,
                    )
                rec = recp.tile([P, QROWS], F32, tag="rec")
                nc.vector.reciprocal_approx_fast(rec[:], pss[:, :QROWS])
                # pss = 16*sum(exp): fold the /16 back via the 16.0 scalar
                nc.vector.scalar_tensor_tensor(
                    xattnT[:, h, :], pso[:, :QROWS], 16.0, rec[:],
                    mybir.AluOpType.mult, mybir.AluOpType.mult,
                )

            # K + V first, then Q, then attention (scores one head ahead)
            with tc.tile_pool(name="wvp", bufs=1, side="left") as wvp:
                wvsb = wvp.tile([P, DT, KVD], BF16, tag="wvsb")
                nc.sync.dma_start(wvsb[:], wv_d[:])
                for m in range(c.KVH if _ph("C") else 0):
                    kproj(m)
                for rt in range(KT if _ph("C") else 0):
                    psv = ps_tile()
                    for dt in range(DT):
                        nc.tensor.matmul(
                            psv[:, :KVD],
                            xkvT[:, dt, rt * P : (rt + 1) * P],
                            wvsb[:, dt, :],
                            start=(dt == 0),
                            stop=(dt == DT - 1),
                        )
                    nc.vector.tensor_add(vN[:, rt, :], psv[:, :KVD], bvbc[:])

            for m in range(c.H if _ph("C") else 0):
                qproj(m)

            if _ph("D"):
                scores_block(0)
                for h in range(c.H):
                    if h + 1 < c.H:
                        scores_block(h + 1)
                    av_ones_drain(h)

        es_qkv.close()  # free kT/vN/qT (right side)

        # attn+mlp residual, lives E -> G
        es_res2 = ExitStack()
        res2p = es_res2.enter_context(tc.tile_pool(name="res2p", bufs=1, side="right"))
        res2 = res2p.tile([P, QT, c.D], F32, tag="res2")

        # ---- phase E: o-proj + residual + rmsnorm2 ----
        es_xm = ExitStack()
        xmp = es_xm.enter_context(tc.tile_pool(name="xmp", bufs=1, side="right"))
        xmT = xmp.tile([P, DT, QROWS], BF16, tag="xmT")

        with tc.tile_pool(name="gpool2", bufs=3, side="left") as gpool2, tc.tile_pool(name="spool2", bufs=4, side="left") as spool2:
            def _norm2_transpose(qt):
                mv = spool2.tile([P, 2], F32, tag="mv2", name="mv")
                nc.vector.bn_aggr(mv[:], stats2[:, qt])
                msq = spool2.tile([P, 1], F32, tag="msq2", name="msq")
                nc.vector.tensor_mul(msq[:], mv[:, 0:1], mv[:, 0:1])
                nc.vector.tensor_add(msq[:], msq[:], mv[:, 1:2])
                srt = spool2.tile([P, 1], F32, tag="srt2", name="srt")
                nc.scalar.activation(srt[:], msq[:], AF.Sqrt, bias=epsc[:])
                rstd = spool2.tile([P, 1], F32, tag="rstd2", name="rstd")
                nc.vector.reciprocal(rstd[:], srt[:])
                xn = gpool2.tile([P, c.D], BF16, tag="xn2", name="xn")
                nc.vector.tensor_scalar_mul(xn[:], res2[:, qt, :], rstd[:])
                for dt in range(DT):
                    tp = psb_tile()
                    nc.tensor.transpose(
                        tp[:], xn[:, dt * P : (dt + 1) * P], idb[:]
                    )
                    nc.scalar.copy(xmT[:, dt, qt * P : (qt + 1) * P], tp[:])

            stats2 = spool2.tile([P, QT, nsub, 6], F32, tag="stats2all")
            for ogi, (os_, ow) in enumerate(OGS if _ph("E") else []):
                pss4 = [ps_tile() for _ in range(QT)]
                for c4 in range(c.H // 4):
                    wot = wo_tiles[(ogi, c4)]
                    for i in range(4):
                        ht = c4 * 4 + i
                        for qt in range(QT):
                            nc.tensor.matmul(
                                pss4[qt][:, :ow],
                                xattnT[:, ht, qt * P : (qt + 1) * P],
                                wot[:, i, :ow],
                                start=(ht == 0),
                                stop=(ht == c.H - 1),
                            )
                for qt in range(QT):
                    nc.vector.tensor_add(
                        res2[:, qt, os_ : os_ + ow],
                        pss4[qt][:, :ow],
                        xq_raw[:, qt, os_ : os_ + ow],
                    )
                    # incremental norm2 stats: OGS chunks == bn subgroups
                    nc.vector.bn_stats(
                        stats2[:, qt, ogi, :], res2[:, qt, os_ : os_ + ow]
                    )
            for qt in range(QT if _ph("E") else 0):
                _norm2_transpose(qt)
            for qt in range(QT if _ph("E") else 0):
                # fold gating: res2 := (res2 - xq_raw)*g + xq_raw, so phase G
                # only needs one fused op per output tile after the last MM
                nc.vector.tensor_sub(
                    res2[:, qt, :], res2[:, qt, :], xq_raw[:, qt, :]
                )
                nc.vector.tensor_scalar_mul(
                    res2[:, qt, :], res2[:, qt, :], gsc[:, qt : qt + 1]
                )
                nc.vector.tensor_add(
                    res2[:, qt, :], res2[:, qt, :], xq_raw[:, qt, :]
                )

        es_attn.close()  # free xattnT
        es_wo.close()    # free wo tiles
        es_bt.close()    # free xkvT/xqT/cos/sin (held through D for Q-proj)
        es_xq.close()    # free xq_raw (gating already folded into res2)

        # ---- phase F: mlp gate/up ----
        es_act = ExitStack()
        actp = es_act.enter_context(tc.tile_pool(name="actp", bufs=1, side="left"))
        actT = actp.tile([P, c.FFT, QROWS], BF16, tag="actT")

        es_wd = ExitStack()  # down-proj weight stream: spans F (prefetch) + G
        wstr4 = es_wd.enter_context(tc.tile_pool(name="wstr4", bufs=6, side="left"))
        wd_pre = []
        for pi in range(2 if _ph("G") else 0):
            wdt = wstr4.tile([P, 4, OW], BF16, tag="wdt")
            nc.gpsimd.dma_start(wdt[:], wd_d[0, pi])
            wd_pre.append(wdt)

        with tc.tile_pool(name="wstr3", bufs=6, side="left") as wstr3, tc.tile_pool(name="fpool", bufs=3, side="left") as fpool:
            for g in range(c.FFG if _ph("F") else 0):
                psg = [ps_tile() for _ in range(4)]
                for d4 in range(DT // 4):
                    wgt = wstr3.tile([P, 4, 512], BF16, tag="wgut")
                    nc.sync.dma_start(wgt[:], wg_d[g, d4])
                    for i in range(4):
                        dt = d4 * 4 + i
                        for s in range(4):
                            nc.tensor.matmul(
                                psg[s][:, :QROWS],
                                wgt[:, i, s * P : (s + 1) * P],
                                xmT[:, dt, :],
                                start=(dt == 0),
                                stop=(dt == DT - 1),
                            )
                silu = fpool.tile([P, 4, QROWS], F32, tag="silu")
                for s in range(4):
                    # silu(x) = x * sigmoid(x) (Silu isn't in CoreSim)
                    nc.scalar.activation(silu[:, s, :], psg[s][:, :QROWS], AF.Sigmoid)
                    nc.vector.tensor_mul(silu[:, s, :], silu[:, s, :], psg[s][:, :QROWS])
                psu = [ps_tile() for _ in range(4)]
                for d4 in range(DT // 4):
                    wut = wstr3.tile([P, 4, 512], BF16, tag="wgut")
                    nc.sync.dma_start(wut[:], wu_d[g, d4])
                    for i in range(4):
                        dt = d4 * 4 + i
                        for s in range(4):
                            nc.tensor.matmul(
                                psu[s][:, :QROWS],
                                wut[:, i, s * P : (s + 1) * P],
                                xmT[:, dt, :],
                                start=(dt == 0),
                                stop=(dt == DT - 1),
                            )
                for s in range(4):
                    nc.vector.tensor_mul(
                        actT[:, g * 4 + s, :], silu[:, s, :], psu[s][:, :QROWS]
                    )

        es_xm.close()  # free xmT

        # ---- phase G: down-proj + residual + gating + output ----
        with tc.tile_pool(name="opool", bufs=3, side="left") as opool:
            for ogi, (os_, ow) in enumerate(OGS if _ph("G") else []):
                psd = [ps_tile() for _ in range(QT)]
                for f4 in range(c.FFT // 4):
                    if ogi == 0 and f4 < 2:
                        wdt = wd_pre[f4]
                    else:
                        wdt = wstr4.tile([P, 4, OW], BF16, tag="wdt")
                        nc.sync.dma_start(wdt[:], wd_d[ogi, f4])
                    for i in range(4):
                        ffp = f4 * 4 + i
                        for qt in range(QT):
                            nc.tensor.matmul(
                                psd[qt][:, :ow],
                                actT[:, ffp, qt * P : (qt + 1) * P],
                                wdt[:, i, :ow],
                                start=(ffp == 0),
                                stop=(ffp == c.FFT - 1),
                            )
                for qt in range(QT):
                    t1 = opool.tile([P, 512], F32, tag="updt")
                    nc.vector.scalar_tensor_tensor(
                        t1[:, :ow],
                        psd[qt][:, :ow],
                        gsc[:, qt : qt + 1],
                        res2[:, qt, os_ : os_ + ow],
                        mybir.AluOpType.mult,
                        mybir.AluOpType.add,
                    )
                    nc.sync.dma_start(
                        oupd_d[qt * P : (qt + 1) * P, os_ : os_ + ow], t1[:, :ow]
                    )

        es_wd.close()
        es_act.close()
        es_res2.close()
    return nc


# ---------------- host side ----------------


def _bf(x):
    return np.ascontiguousarray(x.astype(BF16NP))


def _f32(x):
    return np.ascontiguousarray(x, dtype=np.float32)


def prep_shared(c: Cfg, Wq, bq, Wk, bk, Wv, bv, Wo, w_gate, w_up, w_down, ln1_w, ln2_w):
    """Host-side weight folding + tiling (exact fp32 math, then bf16 cast)."""
    DT, FFT, FFG, KVD = c.DT, c.FFT, c.FFG, c.KVH * c.HD
    OGS = _chunks(c.D, 512)
    OG, OW = len(OGS), OGS[0][1]
    Wqf = _f32(Wq) * _f32(ln1_w)[:, None]
    Wkf = _f32(Wk) * _f32(ln1_w)[:, None]
    Wvf = _f32(Wv) * _f32(ln1_w)[:, None]
    Wgf = _f32(w_gate) * _f32(ln2_w)[:, None]
    Wuf = _f32(w_up) * _f32(ln2_w)[:, None]

    perm = np.zeros((P, P), np.float32)
    half = c.HD // 2
    perm[np.arange(half) + half, np.arange(half)] = -1.0
    perm[np.arange(half), np.arange(half) + half] = 1.0

    # tri[k, q] = 1 if k <= q (keep) else 0, for the diagonal 128x128 block
    tri = np.triu(np.ones((P, P), np.float32))

    return dict(
        wq=_bf(Wqf.reshape(DT, P, c.H, c.HD).transpose(2, 1, 0, 3)),
        wk=_bf(Wkf.reshape(DT, P, c.KVH, c.HD).transpose(2, 1, 0, 3)),
        wv=_bf(Wvf.reshape(DT, P, KVD).transpose(1, 0, 2)),
        # wo[ogi, c4, p, i, col] = Wo[(c4*4+i)*128+p, ogi*512+col]
        wo=_bf(_f32(Wo).reshape(c.H // 4, 4, P, OG, OW).transpose(3, 0, 2, 1, 4)),
        # wg[g, d4, p, i, col] = Wgf[(d4*4+i)*128+p, g*512+col]
        wg=_bf(Wgf.reshape(DT // 4, 4, P, FFG, 512).transpose(3, 0, 2, 1, 4)),
        wu=_bf(Wuf.reshape(DT // 4, 4, P, FFG, 512).transpose(3, 0, 2, 1, 4)),
        # wd[ogi, f4, p, i, col] = w_down[(f4*4+i)*128+p, ogi*512+col]
        wd=_bf(_f32(w_down).reshape(FFT // 4, 4, P, OG, OW).transpose(3, 0, 2, 1, 4)),
        bq=_f32(bq).reshape(c.H, P, 1),
        bk=_f32(bk).reshape(c.KVH, P, 1),
        bv=_f32(bv).reshape(1, KVD),
        id_f=np.eye(P, dtype=np.float32),
        id_b=np.eye(P, dtype=np.float32).astype(BF16NP),
        perm=perm.astype(BF16NP),
        ones16=np.full((P, 2, P), 16.0, np.float32).astype(F8NP),
        tri=tri.astype(F8NP),
    )


def prep_core(c: Cfg, shared, hid_b, idx_b, g_b, cos_b, sin_b, h):
    """Per-core inputs for core handling query-half h of one batch."""
    QROWS, QT, KT = c.QROWS, c.QT, c.KT
    idx32 = idx_b.astype(np.int32)
    # permute keys so this core's own query half comes first; block-causal
    # masking is then uniform: tile j<4 affects only cols >= j*128 with a
    # triangular diagonal block; tiles j>=4 are all-or-nothing via biasj
    kperm = np.concatenate(
        [np.arange(h * QROWS, (h + 1) * QROWS),
         np.arange(0, h * QROWS), np.arange((h + 1) * QROWS, c.KSEL)]
    )
    idx32 = idx32[kperm]
    # exp bias per key tile: -1 everywhere (overflow guard, cancels in the
    # normalization); other-half tiles fully masked for h=0 cores
    biasj = np.full((P, KT), -1.0, np.float32)
    if h == 0:
        biasj[:, 4:] = -30001.0
    m = dict(
        hid=_f32(hid_b),
        idx_kv=np.ascontiguousarray(idx32.reshape(KT, P).T),
        gsc=np.ascontiguousarray(
            _f32(g_b[h * QROWS : (h + 1) * QROWS]).reshape(QT, P).T
        ),
        cosb=_f32(cos_b),
        sinb=_f32(sin_b),
        biasj=biasj,
    )
    m.update(shared)
    return m


_NC_CACHE = {}


def _get_nc(c: Cfg):
    key = c
    if key not in _NC_CACHE:
        nc = bacc.Bacc()
        emit(nc, c)
        nc.compile()
        _NC_CACHE[key] = nc
    return _NC_CACHE[key]


_RUN_CACHE = {}


def _run_spmd_cached(c: Cfg, nc, in_maps):
    """run_bass_via_pjrt equivalent with a cached jitted executable.

    run_bass_kernel_spmd rebuilds its jit closure per call, so every kernel()
    invocation would re-trace + recompile (~40s).  Build the shard_map jit
    once per config and reuse it; repeat calls only pay host->device
    transfer + execution.
    """
    import jax
    import numpy as np
    from jax.sharding import Mesh, PartitionSpec
    from jax.experimental.shard_map import shard_map
    from concourse import bass2jax
    from concourse.bass2jax import _bass_exec_p, install_neuronx_cc_hook

    n_cores = len(in_maps)
    key = (c, n_cores)
    if key not in _RUN_CACHE:
        install_neuronx_cc_hook()
        partition_name = (
            nc.partition_id_tensor.name if nc.partition_id_tensor else None
        )
        in_names, out_names, out_avals = [], [], []
        for alloc in nc.m.functions[0].allocations:
            if not isinstance(alloc, mybir.MemoryLocationSet):
                continue
            name = alloc.memorylocations[0].name
            if alloc.kind == "ExternalInput":
                if name != partition_name:
                    in_names.append(name)
            elif alloc.kind == "ExternalOutput":
                out_names.append(name)
                out_avals.append(
                    jax.core.ShapedArray(
                        tuple(alloc.tensor_shape), mybir.dt.np(alloc.dtype)
                    )
                )
        n_params = len(in_names)
        all_in = list(in_names) + list(out_names)
        if partition_name is not None:
            all_in.append(partition_name)

        def _body(*flat):
            operands = list(flat)
            if partition_name is not None:
                operands.append(bass2jax.partition_id_tensor())
            return tuple(
                _bass_exec_p.bind(
                    *operands,
                    out_avals=tuple(out_avals),
                    in_names=tuple(all_in),
                    out_names=tuple(out_names),
                    lowering_input_output_aliases=(),
                    sim_require_finite=True,
                    sim_require_nnan=True,
                    nc=nc,
                )
            )

        devices = jax.devices()[:n_cores]
        mesh = Mesh(np.asarray(devices), ("core",))
        n_outs = len(out_avals)
        sharded = jax.jit(
            shard_map(
                _body,
                mesh=mesh,
                in_specs=(PartitionSpec("core"),) * (n_params + n_outs),
                out_specs=(PartitionSpec("core"),) * n_outs,
                check_rep=False,
            ),
            keep_unused=True,
        )
        zeros = [
            np.zeros((n_cores * a.shape[0], *a.shape[1:]), a.dtype)
            for a in out_avals
        ]
        _RUN_CACHE[key] = (sharded, in_names, out_names, out_avals, zeros)

    sharded, in_names, out_names, out_avals, zeros = _RUN_CACHE[key]
    concat_in = [
        np.concatenate([np.asarray(in_maps[ci][nm]) for ci in range(n_cores)], axis=0)
        for nm in in_names
    ]
    out_arrs = sharded(*concat_in, *zeros)
    return [
        {
            name: np.asarray(out_arrs[i]).reshape(n_cores, *out_avals[i].shape)[ci]
            for i, name in enumerate(out_names)
        }
        for ci in range(n_cores)
    ]


def assemble_output(inputs, res):
    """Build the full [B, T, D] output from per-core result maps."""
    c = FULL
    hidden_states = np.asarray(inputs["hidden_states"])
    topk = np.asarray(inputs["topk_indices"])
    B = hidden_states.shape[0]
    final = np.ascontiguousarray(hidden_states, dtype=np.float32).copy()
    for ci in range(2 * B):
        b, h = ci // 2, ci % 2
        sel = topk[b, h * c.QROWS : (h + 1) * c.QROWS].astype(np.int64)
        final[b, sel] = res[ci]["out_upd"]
    return final


def kernel(
    hidden_states,
    topk_indices,
    gating_scores,
    cos,
    sin,
    Wq,
    bq,
    Wk,
    bk,
    Wv,
    bv,
    Wo,
    w_gate,
    w_up,
    w_down,
    ln1_w,
    ln2_w,
):
    c = FULL
    B = hidden_states.shape[0]
    hidden_states = np.asarray(hidden_states)
    topk_indices = np.asarray(topk_indices)
    shared = prep_shared(
        c, Wq, bq, Wk, bk, Wv, bv, Wo, w_gate, w_up, w_down, ln1_w, ln2_w
    )
    in_maps = []
    for b in range(B):
        for h in range(2):
            in_maps.append(
                prep_core(
                    c,
                    shared,
                    hidden_states[b],
                    topk_indices[b],
                    np.asarray(gating_scores)[b],
                    np.asarray(cos)[b],
                    np.asarray(sin)[b],
                    h,
                )
            )
    nc = _get_nc(c)
    res = _run_spmd_cached(c, nc, in_maps)

    return assemble_output(
        dict(hidden_states=hidden_states, topk_indices=topk_indices), res
    )
